# revision 12
# baseline (speedup 1.0000x reference)
"""Multi-head self-attention (RoPE, causal) Trainium2 kernel, 8-way sharded.

Sharding: data-parallel over batch (B=2) x tensor-parallel over head groups
(16 heads -> 4 groups of 4). Core c handles batch c//4, heads 4*(c%4)..+4.
Each core computes q/k/v projections for its heads, RoPE, causal-softmax
attention, and a Megatron-style row-parallel partial of the output
projection; the host sums the 4 partials per batch.

Device dataflow (all matmul operands bf16, accumulation f32 in PSUM):
- scores are computed transposed (scores^T[kpos, q]) per 128-row kv strip,
  exp'd in one Activation op per strip into a bf16 p tile, causal mask via
  a bf16 elementwise multiply on the diagonal block only.
- attn@V runs with queries on PSUM partitions: per (strip, q-tile) a
  [128q x 65] matmul accumulates p^T V (the 65th V column is ones, so the
  softmax denominator accumulates alongside). This halves the PE column
  count vs. streaming q on the free axis, and normalization becomes a
  native per-partition tensor_scalar multiply on the gpsimd engine.
- per-head-pair PE transposes restore the [channels, q] layout the output
  projection needs as its stationary operand.
- RoPE: rotate_half is a PE permutation matmul; the sign lives in the sin
  table; the elementwise combine runs on DVE (bf16 ops get the fast DVE
  modes).
- x is fed pre-transposed [C, T] bf16 from the host. Attention runs in two
  query halves so attention starts while input DMA/projections continue,
  and the output projection overlaps the second attention half; projection
  and output work is woven into the attention strip loop as PE filler.
"""
import sys
for _p in ("/opt/trn_rl_repo",):
    if _p not in sys.path:
        sys.path.insert(0, _p)

import numpy as np
from contextlib import ExitStack

import concourse.bacc as bacc
import concourse.mybir as mybir
import concourse.tile as tile
from concourse.bass_utils import run_bass_kernel_spmd

F32 = mybir.dt.float32
F32R = mybir.dt.float32r
BF16 = mybir.dt.bfloat16
AF = mybir.ActivationFunctionType

B, T, C = 2, 2048, 1024
H, Dh = 16, 64
HL = 4                      # heads per core
CK = C // 128               # 8 contraction k-tiles for projections
TTL = T // 128              # 16 T-tiles / kv k-tiles
HT = T // 2                 # 1024, the attention q-half width
N_CORES = 8


def build_nc():
    nc = bacc.Bacc("TRN2", target_bir_lowering=False, debug=False, num_devices=N_CORES)

    xt = nc.declare_dram_parameter("xt", [C, T], BF16, isOutput=False)
    wqkv = nc.declare_dram_parameter("wqkv", [C, 4 * 128 + HL * Dh], BF16, isOutput=False)
    wo = nc.declare_dram_parameter("wo", [HL * Dh, C], BF16, isOutput=False)
    cosT = nc.declare_dram_parameter("cosT", [128, T], BF16, isOutput=False)
    sinT = nc.declare_dram_parameter("sinT", [128, T], F32R, isOutput=False)
    maskT = nc.declare_dram_parameter("maskT", [128, 128], BF16, isOutput=False)
    identT = nc.declare_dram_parameter("identT", [128, 128], BF16, isOutput=False)
    ones4 = nc.declare_dram_parameter("ones4", [128, TTL * HL], BF16, isOutput=False)
    rotT = nc.declare_dram_parameter("rotT", [128, 128], BF16, isOutput=False)
    out = nc.declare_dram_parameter("out", [T, C], F32, isOutput=True)

    with nc.allow_low_precision("bf16 attention pipeline"), \
         tile.TileContext(nc) as tc, ExitStack() as octx:
        consts = octx.enter_context(tc.tile_pool(name="consts", bufs=1))
        v_pool = octx.enter_context(tc.tile_pool(name="v", bufs=1))
        qkt_pool = octx.enter_context(tc.tile_pool(name="qkt", bufs=1))
        ao_pool = octx.enter_context(tc.tile_pool(name="ao", bufs=1))
        p_pool = octx.enter_context(tc.tile_pool(name="pb", bufs=2))
        avn_pool = octx.enter_context(tc.tile_pool(name="avnp", bufs=2))
        rec_pool = octx.enter_context(tc.tile_pool(name="recp", bufs=4))
        wo_pool = octx.enter_context(tc.tile_pool(name="wop", bufs=1))
        sc_ps = octx.enter_context(tc.tile_pool(name="scps", bufs=2, space="PSUM"))
        av_ps = octx.enter_context(tc.tile_pool(name="avps", bufs=2, space="PSUM"))

        mask_t = consts.tile([128, 128], BF16, tag="mask")
        ident_t = consts.tile([128, 128], BF16, tag="ident")
        rotT_t = consts.tile([128, 128], BF16, tag="rotT")

        vext_t = v_pool.tile([128, TTL, HL, Dh + 1], BF16, tag="vext", name="vext")
        vext = [vext_t[:, t_] for t_ in range(TTL)]
        # qkt[mt][half]: mt 0=Q heads01, 1=K heads01, 2=Q heads23, 3=K heads23
        qkt = [[qkt_pool.tile([128, HT], BF16, tag=f"qkt{m}_{hf}", name=f"qkt{m}_{hf}")
                for hf in range(2)] for m in range(4)]
        # ao[pair]: [128 ch (2 heads x 64), T] attention output, transposed
        ao = [ao_pool.tile([128, T], BF16, tag=f"ao{i}", name=f"ao{i}") for i in range(2)]
        wo_t = [wo_pool.tile([128, C], BF16, tag=f"wo{i}", name=f"wo{i}")
                for i in range(2)]

        def attn_unit(h, half, fillers=(), per_qt_sink=None):
            """scores^T/exp/mask + [q,ch]-oriented attn@V for head h, query
            half `half`. `fillers` are independent emission closures injected
            one-per-strip to keep PE fed while the softmax pipeline runs.
            `per_qt_sink(qt)` (if set) is called right after q-tile qt of
            this head is drained+transposed (used on the last head to
            pipeline the output projection into the tail).

            p strips persist in SBUF for the whole head-half; each q-tile's
            attn@V accumulation runs as one contiguous burst over the strips
            (PSUM allows only one pending accumulation group per 2KB bank)."""
            fillers = list(fillers)
            hp, hl = h // 2, h % 2
            qrmt, krmt = (0, 1) if h < 2 else (2, 3)
            pr = 64 * hl
            q_lo = HT * half
            qt0 = 8 * half
            n_strips = 8 if half == 0 else 16
            per_qt = per_qt_sink is not None
            strips = {}

            if hl == 0:
                avn = avn_pool.tile([128, 8, 128], BF16, tag="avn", name="avn")
                state["avn"] = avn
            else:
                avn = state["avn"]

            def transpose_qt(lqt):
                """[128 q, 128 ch] -> ao[hp][:, qcols] via PE transpose.
                tt comes from the projection-phase PSUM pool (its tiles
                retire immediately, unlike the av accumulators)."""
                tt = state["tpool"].tile([128, 128], BF16, tag=state["ttag"], name="tt")
                nc.tensor.transpose(tt[:], avn[:, lqt, :], ident_t[:])
                qtg = qt0 + lqt
                nc.vector.tensor_copy(ao[hp][:, 128 * qtg:128 * (qtg + 1)], tt[:])

            def emit_burst(qt):
                lqt = qt - qt0
                av = av_ps.tile([128, Dh + 1], F32, tag="av", name="av")
                for m2 in range(qt + 1):
                    p_, cs_ = strips[m2]
                    lq = 128 * qt - cs_
                    nc.tensor.matmul(av[:], p_[:, lq:lq + 128], vext[m2][:, h, :],
                                     start=(m2 == 0), stop=(m2 == qt))
                rec = rec_pool.tile([128, 1], F32, tag="rec", name="rec")
                nc.vector.reciprocal(rec[:], av[:, Dh:Dh + 1])
                # normalize out of PSUM into avn (DVE: gpsimd cannot touch PSUM)
                nc.vector.tensor_scalar_mul(
                    avn[:, lqt, pr:pr + 64], av[:, 0:Dh], rec[:])
                if per_qt:
                    transpose_qt(lqt)
                    per_qt_sink(qt)

            pending = None
            for m in range(n_strips):
                cs = max(q_lo, 128 * m)
                W = q_lo + HT - cs
                kr_t = qkt[krmt][m // 8]
                kc = 128 * m - HT * (m // 8)
                sc = sc_ps.tile([128, W], F32, tag="sc", name="sc")
                j = 0
                while 512 * j < W:
                    n = min(512, W - 512 * j)
                    qc = (cs - q_lo) + 512 * j
                    nc.tensor.matmul(
                        sc[:, 512 * j:512 * j + n],
                        kr_t[pr:pr + 64, kc:kc + 128],
                        qkt[qrmt][half][pr:pr + 64, qc:qc + n],
                        start=True, stop=True)
                    j += 1
                p = p_pool.tile([128, W], BF16, tag=f"p{m}", name=f"p{m}")
                nc.scalar.activation(p[:], sc[:, 0:W], AF.Exp, scale=0.125)
                if cs == 128 * m:
                    # gpsimd: all-SBUF bf16, keeps DVE free for PSUM drains
                    nc.gpsimd.tensor_mul(p[:, 0:128], p[:, 0:128], mask_t[:])
                strips[m] = (p, cs)
                if pending is not None:
                    emit_burst(pending)
                    pending = None
                if m >= qt0:
                    pending = m
                if m >= 1 and fillers:
                    fillers.pop(0)()
            if pending is not None:
                emit_burst(pending)
            if hl == 1 and not per_qt:
                for lqt in range(8):
                    transpose_qt(lqt)
            for f in fillers:
                f()

        state = {"avn": None, "tpool": None, "ttag": None}

        with tc.tile_pool(name="xtp", bufs=1) as xt_pool, \
             tc.tile_pool(name="wqkp", bufs=1) as wqk_pool, \
             tc.tile_pool(name="ropetab", bufs=1) as rtab_pool, \
             tc.tile_pool(name="ropetmp", bufs=2) as rtmp_pool, \
             tc.tile_pool(name="projps", bufs=2, space="PSUM") as proj_ps:
            state["tpool"], state["ttag"] = proj_ps, "pp"

            wqkv_t = [wqk_pool.tile([128, 512 + HL * Dh], BF16, tag=f"wqkv{k}", name=f"wqkv{k}")
                      for k in range(CK)]
            wqk_t = [w[:, 0:512] for w in wqkv_t]
            wv_t = [w[:, 512:512 + HL * Dh] for w in wqkv_t]

            xt_t = [xt_pool.tile([128, T], BF16, tag=f"xt{k}", name=f"xt{k}")
                    for k in range(CK)]

            cos_t = rtab_pool.tile([128, T], BF16, tag="cos")
            sin_t = rtab_pool.tile([128, T], F32R, tag="sin")

            # DMA emission order tracks the critical path: the first
            # projection group needs wqkv[k] + xt[k][:, 0:512] for all k.
            for k in range(CK):
                nc.sync.dma_start(wqkv_t[k][:], wqkv[128 * k:128 * (k + 1), :])
                nc.sync.dma_start(xt_t[k][:, 0:512], xt[128 * k:128 * (k + 1), 0:512])
            nc.sync.dma_start(cos_t[:, 0:HT], cosT[:, 0:HT])
            nc.sync.dma_start(sin_t[:, 0:HT], sinT[:, 0:HT])
            nc.sync.dma_start(mask_t[:], maskT[:])
            nc.sync.dma_start(rotT_t[:], rotT[:])
            nc.sync.dma_start(ident_t[:], identT[:])
            for k in range(CK):
                nc.sync.dma_start(xt_t[k][:, 512:HT], xt[128 * k:128 * (k + 1), 512:HT])
            for i in range(2):
                nc.sync.dma_start(wo_t[i][:], wo[128 * i:128 * (i + 1), :])
            nc.sync.dma_start(cos_t[:, HT:T], cosT[:, HT:T])
            nc.sync.dma_start(sin_t[:, HT:T], sinT[:, HT:T])

            def xt_dma(hf):
                for k in range(CK):
                    nc.sync.dma_start(xt_t[k][:, HT * hf:HT * (hf + 1)],
                                      xt[128 * k:128 * (k + 1), HT * hf:HT * (hf + 1)])

            rope_pending = []

            def emit_rope(m, n):
                """rotate-half via a PE permutation matmul, then the cos/sin
                elementwise combine on DVE. Emitted one projection group late
                so the PSUM->SBUF drain has completed."""
                dst = qkt[m][n // 2]
                src = dst[:, 512 * (n % 2):512 * (n % 2 + 1)]
                rps = sc_ps.tile([128, 512], F32, tag="sc", name="rps")
                nc.tensor.matmul(rps[:], rotT_t[:], src, start=True, stop=True)
                rot = rtmp_pool.tile([128, 512], BF16, tag="rot", name="rot")
                nc.vector.tensor_mul(rot[:], rps[:].bitcast(F32R),
                                     sin_t[:, 512 * n:512 * (n + 1)])
                nc.gpsimd.tensor_mul(src, src, cos_t[:, 512 * n:512 * (n + 1)])
                nc.vector.tensor_add(src, src, rot[:])

            def flush_rope():
                while rope_pending:
                    emit_rope(*rope_pending.pop(0))

            def proj_group(m, n):
                pp = proj_ps.tile([128, 512], F32, tag="pp", name="pp")
                for k in range(CK):
                    nc.tensor.matmul(pp[:], wqk_t[k][:, 128 * m:128 * (m + 1)],
                                     xt_t[k][:, 512 * n:512 * (n + 1)],
                                     start=(k == 0), stop=(k == CK - 1))
                dst = qkt[m][n // 2]
                nc.scalar.copy(dst[:, 512 * (n % 2):512 * (n % 2 + 1)], pp[:])
                pending = rope_pending[:]
                rope_pending.clear()
                rope_pending.append((m, n))
                for pmn in pending:
                    emit_rope(*pmn)

            def vproj_tile(t_):
                flush_rope()
                vp = proj_ps.tile([128, HL * Dh], F32, tag="pp", name="vp")
                for k in range(CK):
                    nc.tensor.matmul(vp[:], xt_t[k][:, 128 * t_:128 * (t_ + 1)], wv_t[k][:],
                                     start=(k == 0), stop=(k == CK - 1))
                nc.scalar.copy(
                    vext[t_][:, :, 0:Dh],
                    vp[:].rearrange("p (h d) -> p h d", h=HL))
                nc.sync.dma_start(
                    vext[t_][:, :, Dh:Dh + 1],
                    ones4[:, HL * t_:HL * (t_ + 1)].rearrange("p (h x) -> p h x", x=1))

            # heads01 projections + V for the first query half, then attention
            # units with the remaining projection work injected between strips
            # (PE executes in emission order, so attention must be emitted as
            # soon as its dependencies are, with later work woven in as filler)
            def pg(m, n):
                return lambda: proj_group(m, n)

            def vt(t_):
                return lambda: vproj_tile(t_)

            proj_group(0, 0)
            proj_group(1, 0)
            proj_group(0, 1)
            proj_group(1, 1)
            for t_ in range(0, 4):
                vproj_tile(t_)
            attn_unit(0, 0, [vt(4), vt(5), vt(6), vt(7), pg(2, 0), pg(2, 1)])
            xt_dma(1)
            attn_unit(1, 0, [pg(3, 0), pg(3, 1), pg(0, 2), pg(1, 2), vt(8), vt(9)])
            attn_unit(2, 0, [vt(10), vt(11), pg(0, 3), pg(1, 3), vt(12), vt(13)])
            attn_unit(3, 0, [vt(14), vt(15), pg(2, 2), pg(2, 3), pg(3, 2), pg(3, 3)])
            flush_rope()

        with tc.tile_pool(name="outsb", bufs=3) as out_pool, \
             tc.tile_pool(name="opps", bufs=2, space="PSUM") as op_ps:
            state["tpool"], state["ttag"] = op_ps, "op"

            def outproj_tile(t_):
                osb = out_pool.tile([128, C], F32, tag="osb", name="osb")
                for n in range(2):
                    op = op_ps.tile([128, 512], F32, tag="op", name="op")
                    nc.tensor.matmul(op[:],
                                     ao[0][:, 128 * t_:128 * (t_ + 1)],
                                     wo_t[0][:, 512 * n:512 * (n + 1)],
                                     start=True, stop=False)
                    nc.tensor.matmul(op[:],
                                     ao[1][:, 128 * t_:128 * (t_ + 1)],
                                     wo_t[1][:, 512 * n:512 * (n + 1)],
                                     start=False, stop=True)
                    nc.vector.tensor_copy(osb[:, 512 * n:512 * (n + 1)], op[:])
                nc.sync.dma_start(out[128 * t_:128 * (t_ + 1), :], osb[:])

            def ot(t_):
                return lambda: outproj_tile(t_)

            attn_unit(0, 1, [ot(0), ot(1), ot(2)])
            attn_unit(1, 1, [ot(3), ot(4)])
            attn_unit(2, 1, [ot(5), ot(6), ot(7)])
            attn_unit(3, 1, per_qt_sink=lambda qt: outproj_tile(qt))

    nc.finalize()
    return nc


_NC = None


def _get_nc():
    global _NC
    if _NC is None:
        _NC = build_nc()
    return _NC


def _host_tables():
    import ml_dtypes
    bf16 = ml_dtypes.bfloat16
    inv_freq = 1.0 / (10000.0 ** (np.arange(0, Dh, 2, dtype=np.float32) / Dh))  # [32]
    t = np.arange(T, dtype=np.float32)
    freqs = t[:, None] * inv_freq[None, :]                  # [T, 32]
    emb = np.concatenate([freqs, freqs], axis=-1)           # [T, 64]
    cos = np.cos(emb).T.astype(np.float32)                  # [64, T]
    sin = np.sin(emb).T.astype(np.float32)                  # [64, T]
    sin_signed = sin.copy()
    sin_signed[0:32, :] *= -1.0                             # rotate_half sign fold
    cosT = np.concatenate([cos, cos], axis=0).astype(bf16)  # [128, T] two head-halves
    sinT = np.ascontiguousarray(np.concatenate([sin_signed, sin_signed], axis=0))
    maskT = np.triu(np.ones((128, 128), np.float32)).astype(bf16)  # keep where k <= q
    identT = np.eye(128, dtype=np.float32).astype(bf16)
    sigma = np.empty(64, np.int64)
    sigma[0:32] = 2 * np.arange(32) + 1
    sigma[32:64] = 2 * np.arange(32)
    R = np.zeros((128, 128), np.float32)
    for hh in range(2):
        for d in range(64):
            R[64 * hh + d, 64 * hh + sigma[d]] = 1.0
    rotT = np.ascontiguousarray(R.T).astype(bf16)
    return cosT, sinT, maskT, identT, rotT


def kernel(x, w_qkv, w_out):
    import ml_dtypes
    bf16 = ml_dtypes.bfloat16
    x = np.asarray(x, dtype=np.float32)
    w_qkv = np.asarray(w_qkv, dtype=np.float32)
    w_out = np.asarray(w_out, dtype=np.float32)
    nc = _get_nc()
    cosT, sinT, maskT, identT, rotT = _host_tables()
    ones4 = np.ones((128, TTL * HL), bf16)

    in_maps = []
    for core in range(N_CORES):
        b = core // 4
        g = core % 4
        heads = [4 * g + l for l in range(HL)]
        qcols = [w_qkv[:, 64 * h:64 * (h + 1)] for h in heads]
        kcols = [w_qkv[:, C + 64 * h:C + 64 * (h + 1)] for h in heads]
        vcols = [w_qkv[:, 2 * C + 64 * h:2 * C + 64 * (h + 1)] for h in heads]
        # m-tiles: Q01 | K01 | Q23 | K23
        wqkv_loc = np.concatenate(
            [qcols[0], qcols[1], kcols[0], kcols[1], qcols[2], qcols[3], kcols[2], kcols[3]]
            + vcols, axis=1).astype(bf16)                    # [C, 768]
        wo_loc = np.concatenate([w_out[64 * h:64 * (h + 1), :] for h in heads],
                                axis=0).astype(bf16)
        in_maps.append({
            "xt": np.ascontiguousarray(x[b].T).astype(bf16),  # [C, T]
            "wqkv": wqkv_loc,
            "wo": wo_loc,
            "cosT": cosT, "sinT": sinT, "maskT": maskT,
            "identT": identT, "rotT": rotT, "ones4": ones4,
        })

    res = run_bass_kernel_spmd(nc, in_maps, core_ids=list(range(N_CORES)))
    out_arr = np.zeros((B, T, C), np.float32)
    for core in range(N_CORES):
        out_arr[core // 4] += res.results[core]["out"]
    return out_arr


# revision 15
# speedup vs baseline: 1.0121x; 1.0121x over previous
"""Multi-head self-attention (RoPE, causal) Trainium2 kernel, 8-way sharded.

Sharding: data-parallel over batch (B=2) x tensor-parallel over head groups
(16 heads -> 4 groups of 4). Core c handles batch c//4, heads 4*(c%4)..+4.
Each core computes q/k/v projections for its heads, RoPE, causal-softmax
attention, and a Megatron-style row-parallel partial of the output
projection; the host sums the 4 partials per batch.

Device dataflow (all matmul operands bf16, accumulation f32 in PSUM):
- scores are computed transposed (scores^T[kpos, q]) per 128-row kv strip,
  exp'd in one Activation op per strip into a bf16 p tile that persists for
  the head-half; causal mask is a bf16 multiply on the diagonal block only.
- attn@V runs with queries on PSUM partitions: per q-tile one contiguous
  burst of [128q x 65] matmuls accumulates p^T V over the kv strips (the
  65th V column is ones so the softmax denominator rides along; PSUM allows
  one pending accumulation group per 2KB bank, hence the burst form). This
  halves PE column count vs. streaming q on the free axis, and
  normalization becomes a native per-partition tensor_scalar multiply.
- per-q-tile PE transposes restore the [channels, q] layout the output
  projection needs as its stationary operand.
- RoPE: rotate_half is a PE permutation matmul; the sign lives in the sin
  table; the elementwise combine is split across DVE/gpsimd.
- work is phase-balanced against the Activation engine (exp is ~60us and
  binds the second query half): V projections for the second half and the
  tail head-pair q/k projections are deferred into the second half as PE
  filler, woven between attention strips.
"""
import sys
for _p in ("/opt/trn_rl_repo",):
    if _p not in sys.path:
        sys.path.insert(0, _p)

import numpy as np
from contextlib import ExitStack

import concourse.bacc as bacc
import concourse.mybir as mybir
import concourse.tile as tile
from concourse.bass_utils import run_bass_kernel_spmd

F32 = mybir.dt.float32
F32R = mybir.dt.float32r
BF16 = mybir.dt.bfloat16
AF = mybir.ActivationFunctionType

B, T, C = 2, 2048, 1024
H, Dh = 16, 64
HL = 4                      # heads per core
CK = C // 128               # 8 contraction k-tiles for projections
TTL = T // 128              # 16 T-tiles / kv k-tiles
HT = T // 2                 # 1024, the attention q-half width
N_CORES = 8


def build_nc():
    nc = bacc.Bacc("TRN2", target_bir_lowering=False, debug=False, num_devices=N_CORES)

    xt = nc.declare_dram_parameter("xt", [C, T], BF16, isOutput=False)
    wqkv = nc.declare_dram_parameter("wqkv", [C, 4 * 128 + HL * Dh], BF16, isOutput=False)
    wo = nc.declare_dram_parameter("wo", [HL * Dh, C], BF16, isOutput=False)
    cosT = nc.declare_dram_parameter("cosT", [128, T], BF16, isOutput=False)
    sinT = nc.declare_dram_parameter("sinT", [128, T], F32R, isOutput=False)
    maskT = nc.declare_dram_parameter("maskT", [128, 128], BF16, isOutput=False)
    identT = nc.declare_dram_parameter("identT", [128, 128], BF16, isOutput=False)
    ones4 = nc.declare_dram_parameter("ones4", [128, TTL * HL], BF16, isOutput=False)
    rotT = nc.declare_dram_parameter("rotT", [128, 128], BF16, isOutput=False)
    out = nc.declare_dram_parameter("out", [T, C], F32, isOutput=True)

    with nc.allow_low_precision("bf16 attention pipeline"), \
         tile.TileContext(nc) as tc, ExitStack() as octx:
        pool = lambda *a, **kw: octx.enter_context(tc.tile_pool(*a, **kw))
        consts = pool(name="consts", bufs=1)
        v_pool = pool(name="v", bufs=1)
        qkt_pool = pool(name="qkt", bufs=1)
        ao_pool = pool(name="ao", bufs=1)
        p_pool = pool(name="pb", bufs=2)
        avn_pool = pool(name="avnp", bufs=2)
        rec_pool = pool(name="recp", bufs=4)
        wo_pool = pool(name="wop", bufs=1)
        xt_pool = pool(name="xtp", bufs=1)
        wqk_pool = pool(name="wqkp", bufs=1)
        rtab_pool = pool(name="ropetab", bufs=1)
        rtmp_pool = pool(name="ropetmp", bufs=2)
        out_pool = pool(name="outsb", bufs=3)
        sc_ps = pool(name="scps", bufs=2, space="PSUM")
        av_ps = pool(name="avps", bufs=2, space="PSUM")
        wk_ps = pool(name="wkps", bufs=2, space="PSUM")

        mask_t = consts.tile([128, 128], BF16, tag="mask")
        ident_t = consts.tile([128, 128], BF16, tag="ident")
        rotT_t = consts.tile([128, 128], BF16, tag="rotT")

        vext_t = v_pool.tile([128, TTL, HL, Dh + 1], BF16, tag="vext", name="vext")
        vext = [vext_t[:, t_] for t_ in range(TTL)]
        # qkt[mt][half]: mt 0=Q heads01, 1=K heads01, 2=Q heads23, 3=K heads23
        qkt = [[qkt_pool.tile([128, HT], BF16, tag=f"qkt{m}_{hf}", name=f"qkt{m}_{hf}")
                for hf in range(2)] for m in range(4)]
        # ao[pair]: [128 ch (2 heads x 64), T] attention output, transposed
        ao = [ao_pool.tile([128, T], BF16, tag=f"ao{i}", name=f"ao{i}") for i in range(2)]
        wo_t = [wo_pool.tile([128, C], BF16, tag=f"wo{i}", name=f"wo{i}")
                for i in range(2)]
        wqkv_t = [wqk_pool.tile([128, 512 + HL * Dh], BF16, tag=f"wqkv{k}", name=f"wqkv{k}")
                  for k in range(CK)]
        wqk_t = [w[:, 0:512] for w in wqkv_t]
        wv_t = [w[:, 512:512 + HL * Dh] for w in wqkv_t]
        xt_t = [xt_pool.tile([128, T], BF16, tag=f"xt{k}", name=f"xt{k}")
                for k in range(CK)]
        cos_t = rtab_pool.tile([128, T], BF16, tag="cos")
        sin_t = rtab_pool.tile([128, T], F32R, tag="sin")

        state = {"avn": None}

        # ---- input DMA, ordered by first consumption -------------------
        # first projection groups need wqkv[k][:, 0:256] + xt[k][:, 0:512]
        for k in range(CK):
            nc.sync.dma_start(wqkv_t[k][:, 0:256], wqkv[128 * k:128 * (k + 1), 0:256])
            nc.sync.dma_start(xt_t[k][:, 0:512], xt[128 * k:128 * (k + 1), 0:512])
        nc.sync.dma_start(cos_t[:, 0:HT], cosT[:, 0:HT])
        nc.sync.dma_start(sin_t[:, 0:HT], sinT[:, 0:HT])
        for k in range(CK):   # second x quarter + V weights
            nc.sync.dma_start(xt_t[k][:, 512:HT], xt[128 * k:128 * (k + 1), 512:HT])
            nc.sync.dma_start(wqkv_t[k][:, 512:768], wqkv[128 * k:128 * (k + 1), 512:768])
        nc.sync.dma_start(mask_t[:], maskT[:])
        nc.sync.dma_start(rotT_t[:], rotT[:])
        nc.sync.dma_start(ident_t[:], identT[:])
        for k in range(CK):   # heads-23 q/k weight columns
            nc.sync.dma_start(wqkv_t[k][:, 256:512], wqkv[128 * k:128 * (k + 1), 256:512])
        for i in range(2):
            nc.sync.dma_start(wo_t[i][:], wo[128 * i:128 * (i + 1), :])
        nc.sync.dma_start(cos_t[:, HT:T], cosT[:, HT:T])
        nc.sync.dma_start(sin_t[:, HT:T], sinT[:, HT:T])

        def xt_dma(hf):
            for k in range(CK):
                nc.sync.dma_start(xt_t[k][:, HT * hf:HT * (hf + 1)],
                                  xt[128 * k:128 * (k + 1), HT * hf:HT * (hf + 1)])

        # ---- projections + RoPE ----------------------------------------
        rope_pending = []

        def emit_rope(m, n):
            """rotate-half via a PE permutation matmul, then the cos/sin
            elementwise combine. Emitted one projection group late so the
            PSUM->SBUF drain has completed."""
            dst = qkt[m][n // 2]
            src = dst[:, 512 * (n % 2):512 * (n % 2 + 1)]
            rps = sc_ps.tile([128, 512], F32, tag="sc", name="rps")
            nc.tensor.matmul(rps[:], rotT_t[:], src, start=True, stop=True)
            rot = rtmp_pool.tile([128, 512], BF16, tag="rot", name="rot")
            nc.vector.tensor_mul(rot[:], rps[:].bitcast(F32R),
                                 sin_t[:, 512 * n:512 * (n + 1)])
            nc.gpsimd.tensor_mul(src, src, cos_t[:, 512 * n:512 * (n + 1)])
            nc.vector.tensor_add(src, src, rot[:])

        def flush_rope():
            while rope_pending:
                emit_rope(*rope_pending.pop(0))

        def proj_group(m, n, eng="act"):
            pp = wk_ps.tile([128, 512], F32, tag="pp", name="pp")
            for k in range(CK):
                nc.tensor.matmul(pp[:], wqk_t[k][:, 128 * m:128 * (m + 1)],
                                 xt_t[k][:, 512 * n:512 * (n + 1)],
                                 start=(k == 0), stop=(k == CK - 1))
            dst = qkt[m][n // 2]
            dsl = dst[:, 512 * (n % 2):512 * (n % 2 + 1)]
            if eng == "act":
                nc.scalar.copy(dsl, pp[:])
            else:
                nc.vector.tensor_copy(dsl, pp[:])
            pending = rope_pending[:]
            rope_pending.clear()
            rope_pending.append((m, n))
            for pmn in pending:
                emit_rope(*pmn)

        def vproj_tile(t_, eng="act"):
            flush_rope()
            vp = wk_ps.tile([128, HL * Dh], F32, tag="pp", name="vp")
            for k in range(CK):
                nc.tensor.matmul(vp[:], xt_t[k][:, 128 * t_:128 * (t_ + 1)], wv_t[k][:],
                                 start=(k == 0), stop=(k == CK - 1))
            src = vp[:].rearrange("p (h d) -> p h d", h=HL)
            if eng == "act":
                nc.scalar.copy(vext[t_][:, :, 0:Dh], src)
            else:
                nc.vector.tensor_copy(vext[t_][:, :, 0:Dh], src)
            nc.sync.dma_start(
                vext[t_][:, :, Dh:Dh + 1],
                ones4[:, HL * t_:HL * (t_ + 1)].rearrange("p (h x) -> p h x", x=1))

        # ---- attention ---------------------------------------------------
        def attn_unit(h, half, fillers=(), per_qt_sink=None):
            """scores^T/exp/mask + [q,ch]-oriented attn@V for head h, query
            half `half`. `fillers` are independent emission closures injected
            one-per-strip to keep PE fed while the softmax pipeline runs.
            `per_qt_sink(qt)` (if set) is called right after q-tile qt of
            this head is drained+transposed (used on the last head to
            pipeline the output projection into the tail)."""
            fillers = list(fillers)
            hp, hl = h // 2, h % 2
            qrmt, krmt = (0, 1) if h < 2 else (2, 3)
            pr = 64 * hl
            q_lo = HT * half
            qt0 = 8 * half
            n_strips = 8 if half == 0 else 16
            per_qt = per_qt_sink is not None
            strips = {}

            if hl == 0:
                avn = avn_pool.tile([128, 8, 128], BF16, tag="avn", name="avn")
                state["avn"] = avn
            else:
                avn = state["avn"]

            def transpose_qt(lqt):
                """[128 q, 128 ch] -> ao[hp][:, qcols] via PE transpose.
                tt comes from the shared work PSUM pool (its tiles retire
                immediately, unlike the av accumulators)."""
                tt = wk_ps.tile([128, 128], BF16, tag="pp", name="tt")
                nc.tensor.transpose(tt[:], avn[:, lqt, :], ident_t[:])
                qtg = qt0 + lqt
                nc.vector.tensor_copy(ao[hp][:, 128 * qtg:128 * (qtg + 1)], tt[:])

            def emit_burst(qt):
                lqt = qt - qt0
                av = av_ps.tile([128, Dh + 1], F32, tag="av", name="av")
                for m2 in range(qt + 1):
                    p_, cs_ = strips[m2]
                    lq = 128 * qt - cs_
                    nc.tensor.matmul(av[:], p_[:, lq:lq + 128], vext[m2][:, h, :],
                                     start=(m2 == 0), stop=(m2 == qt))
                rec = rec_pool.tile([128, 1], F32, tag="rec", name="rec")
                nc.vector.reciprocal(rec[:], av[:, Dh:Dh + 1])
                # normalize out of PSUM into avn (DVE: gpsimd cannot touch PSUM)
                nc.vector.tensor_scalar_mul(
                    avn[:, lqt, pr:pr + 64], av[:, 0:Dh], rec[:])
                if per_qt:
                    transpose_qt(lqt)
                    per_qt_sink(qt)

            pending = None
            for m in range(n_strips):
                cs = max(q_lo, 128 * m)
                W = q_lo + HT - cs
                kr_t = qkt[krmt][m // 8]
                kc = 128 * m - HT * (m // 8)
                sc = sc_ps.tile([128, W], F32, tag="sc", name="sc")
                j = 0
                while 512 * j < W:
                    n = min(512, W - 512 * j)
                    qc = (cs - q_lo) + 512 * j
                    nc.tensor.matmul(
                        sc[:, 512 * j:512 * j + n],
                        kr_t[pr:pr + 64, kc:kc + 128],
                        qkt[qrmt][half][pr:pr + 64, qc:qc + n],
                        start=True, stop=True)
                    j += 1
                p = p_pool.tile([128, W], BF16, tag=f"p{m}", name=f"p{m}")
                nc.scalar.activation(p[:], sc[:, 0:W], AF.Exp, scale=0.125)
                if cs == 128 * m:
                    # gpsimd: all-SBUF bf16, keeps DVE free for PSUM drains
                    nc.gpsimd.tensor_mul(p[:, 0:128], p[:, 0:128], mask_t[:])
                strips[m] = (p, cs)
                if pending is not None:
                    emit_burst(pending)
                    pending = None
                if m >= qt0:
                    pending = m
                if m >= 1 and fillers:
                    fillers.pop(0)()
            if pending is not None:
                emit_burst(pending)
            if hl == 1 and not per_qt:
                for lqt in range(8):
                    transpose_qt(lqt)
            for f in fillers:
                f()

        # ---- output projection ------------------------------------------
        def outproj_tile(t_, tail=False):
            osb = out_pool.tile([128, C], F32, tag="osb", name="osb")
            for n in range(2):
                op = wk_ps.tile([128, 512], F32, tag="pp", name="op")
                nc.tensor.matmul(op[:],
                                 ao[0][:, 128 * t_:128 * (t_ + 1)],
                                 wo_t[0][:, 512 * n:512 * (n + 1)],
                                 start=True, stop=False)
                nc.tensor.matmul(op[:],
                                 ao[1][:, 128 * t_:128 * (t_ + 1)],
                                 wo_t[1][:, 512 * n:512 * (n + 1)],
                                 start=False, stop=True)
                if tail and n == 1:
                    # Act is idle in the drain tail; split engines + chunked
                    # DMA to shorten the critical path
                    nc.scalar.copy(osb[:, 512 * n:512 * (n + 1)], op[:])
                else:
                    nc.vector.tensor_copy(osb[:, 512 * n:512 * (n + 1)], op[:])
                if tail:
                    nc.sync.dma_start(out[128 * t_:128 * (t_ + 1), 512 * n:512 * (n + 1)],
                                      osb[:, 512 * n:512 * (n + 1)])
            if not tail:
                nc.sync.dma_start(out[128 * t_:128 * (t_ + 1), :], osb[:])

        def pg(m, n, eng="act"):
            return lambda: proj_group(m, n, eng)

        def vt(t_, eng="act"):
            return lambda: vproj_tile(t_, eng)

        def ot(t_):
            return lambda: outproj_tile(t_)

        # ---- schedule ----------------------------------------------------
        # phase 1: heads01 q/k for both halves + V for the first query half,
        # woven into the half-0 attention units. Act has slack here, so
        # PSUM drains go to Act.
        proj_group(0, 0)
        proj_group(1, 0)
        proj_group(0, 1)
        proj_group(1, 1)
        for t_ in range(0, 4):
            vproj_tile(t_)
        attn_unit(0, 0, [vt(4), vt(5), vt(6), vt(7), pg(2, 0), pg(2, 1)])
        xt_dma(1)
        attn_unit(1, 0, [pg(3, 0), pg(3, 1), pg(0, 2), pg(1, 2), pg(0, 3), pg(1, 3)])
        attn_unit(2, 0, [flush_rope, vt(8), vt(9)])
        attn_unit(3, 0, [vt(10), vt(11)])
        flush_rope()

        # phase 2: Act is saturated by exp; the deferred V projections,
        # heads-23 half-1 q/k projections and the output projection keep PE
        # fed between strips (drains on DVE).
        attn_unit(0, 1, [vt(12, "dve"), vt(13, "dve"), vt(14, "dve"), vt(15, "dve")])
        attn_unit(1, 1, [pg(2, 2, "dve"), pg(2, 3, "dve"), pg(3, 2, "dve"),
                         pg(3, 3, "dve"), flush_rope, ot(0), ot(1)])
        attn_unit(2, 1, [ot(2), ot(3), ot(4), ot(5), ot(6), ot(7)])
        attn_unit(3, 1, per_qt_sink=lambda qt: outproj_tile(qt, tail=(qt >= 14)))

    nc.finalize()
    return nc


_NC = None


def _get_nc():
    global _NC
    if _NC is None:
        _NC = build_nc()
    return _NC


def _host_tables():
    import ml_dtypes
    bf16 = ml_dtypes.bfloat16
    inv_freq = 1.0 / (10000.0 ** (np.arange(0, Dh, 2, dtype=np.float32) / Dh))  # [32]
    t = np.arange(T, dtype=np.float32)
    freqs = t[:, None] * inv_freq[None, :]                  # [T, 32]
    emb = np.concatenate([freqs, freqs], axis=-1)           # [T, 64]
    cos = np.cos(emb).T.astype(np.float32)                  # [64, T]
    sin = np.sin(emb).T.astype(np.float32)                  # [64, T]
    sin_signed = sin.copy()
    sin_signed[0:32, :] *= -1.0                             # rotate_half sign fold
    cosT = np.concatenate([cos, cos], axis=0).astype(bf16)  # [128, T] two head-halves
    sinT = np.ascontiguousarray(np.concatenate([sin_signed, sin_signed], axis=0))
    maskT = np.triu(np.ones((128, 128), np.float32)).astype(bf16)  # keep where k <= q
    identT = np.eye(128, dtype=np.float32).astype(bf16)
    sigma = np.empty(64, np.int64)
    sigma[0:32] = 2 * np.arange(32) + 1
    sigma[32:64] = 2 * np.arange(32)
    R = np.zeros((128, 128), np.float32)
    for hh in range(2):
        for d in range(64):
            R[64 * hh + d, 64 * hh + sigma[d]] = 1.0
    rotT = np.ascontiguousarray(R.T).astype(bf16)
    return cosT, sinT, maskT, identT, rotT


def kernel(x, w_qkv, w_out):
    import ml_dtypes
    bf16 = ml_dtypes.bfloat16
    x = np.asarray(x, dtype=np.float32)
    w_qkv = np.asarray(w_qkv, dtype=np.float32)
    w_out = np.asarray(w_out, dtype=np.float32)
    nc = _get_nc()
    cosT, sinT, maskT, identT, rotT = _host_tables()
    ones4 = np.ones((128, TTL * HL), bf16)

    in_maps = []
    for core in range(N_CORES):
        b = core // 4
        g = core % 4
        heads = [4 * g + l for l in range(HL)]
        qcols = [w_qkv[:, 64 * h:64 * (h + 1)] for h in heads]
        kcols = [w_qkv[:, C + 64 * h:C + 64 * (h + 1)] for h in heads]
        vcols = [w_qkv[:, 2 * C + 64 * h:2 * C + 64 * (h + 1)] for h in heads]
        # m-tiles: Q01 | K01 | Q23 | K23
        wqkv_loc = np.concatenate(
            [qcols[0], qcols[1], kcols[0], kcols[1], qcols[2], qcols[3], kcols[2], kcols[3]]
            + vcols, axis=1).astype(bf16)                    # [C, 768]
        wo_loc = np.concatenate([w_out[64 * h:64 * (h + 1), :] for h in heads],
                                axis=0).astype(bf16)
        in_maps.append({
            "xt": np.ascontiguousarray(x[b].T).astype(bf16),  # [C, T]
            "wqkv": wqkv_loc,
            "wo": wo_loc,
            "cosT": cosT, "sinT": sinT, "maskT": maskT,
            "identT": identT, "rotT": rotT, "ones4": ones4,
        })

    res = run_bass_kernel_spmd(nc, in_maps, core_ids=list(range(N_CORES)))
    out_arr = np.zeros((B, T, C), np.float32)
    for core in range(N_CORES):
        out_arr[core // 4] += res.results[core]["out"]
    return out_arr


# revision 22
# speedup vs baseline: 1.0239x; 1.0116x over previous
"""Multi-head self-attention (RoPE, causal) Trainium2 kernel, 8-way sharded.

Sharding: data-parallel over batch (B=2) x tensor-parallel over head groups
(16 heads -> 4 groups of 4). Core c handles batch c//4, heads 4*(c%4)..+4.
Each core computes q/k/v projections for its heads, RoPE, causal-softmax
attention, and a Megatron-style row-parallel partial of the output
projection; the host sums the 4 partials per batch.

Device dataflow (all matmul operands bf16, accumulation f32 in PSUM):
- scores are computed transposed (scores^T[kpos, q]) per 128-row kv strip,
  exp'd in one Activation op per strip into a bf16 p tile that persists for
  the head-half; causal mask is a bf16 multiply on the diagonal block only.
- attn@V runs with queries on PSUM partitions: per q-tile one contiguous
  burst of [128q x 65] matmuls accumulates p^T V over the kv strips (the
  65th V column is ones so the softmax denominator rides along; PSUM allows
  one pending accumulation group per 2KB bank, hence the burst form). This
  halves PE column count vs. streaming q on the free axis, and
  normalization becomes a native per-partition tensor_scalar multiply.
- per-q-tile PE transposes restore the [channels, q] layout the output
  projection needs as its stationary operand.
- RoPE: rotate_half is a PE permutation matmul; the sign lives in the sin
  table; the elementwise combine is split across DVE/gpsimd.
- work is phase-balanced against the Activation engine (exp is ~60us and
  binds the second query half): V projections for the second half and the
  tail head-pair q/k projections are deferred into the second half as PE
  filler, woven between attention strips.
"""
import sys
for _p in ("/opt/trn_rl_repo",):
    if _p not in sys.path:
        sys.path.insert(0, _p)

import numpy as np
from contextlib import ExitStack

import concourse.bacc as bacc
import concourse.mybir as mybir
import concourse.tile as tile
from concourse.bass_utils import run_bass_kernel_spmd

F32 = mybir.dt.float32
F32R = mybir.dt.float32r
BF16 = mybir.dt.bfloat16
AF = mybir.ActivationFunctionType

B, T, C = 2, 2048, 1024
H, Dh = 16, 64
HL = 4                      # heads per core
CK = C // 128               # 8 contraction k-tiles for projections
TTL = T // 128              # 16 T-tiles / kv k-tiles
HT = T // 2                 # 1024, the attention q-half width
N_CORES = 8


def build_nc():
    nc = bacc.Bacc("TRN2", target_bir_lowering=False, debug=False, num_devices=N_CORES)

    xt = nc.declare_dram_parameter("xt", [C, T], BF16, isOutput=False)
    wqkv = nc.declare_dram_parameter("wqkv", [C, 4 * 128 + HL * Dh], BF16, isOutput=False)
    wo = nc.declare_dram_parameter("wo", [HL * Dh, C], BF16, isOutput=False)
    cosT = nc.declare_dram_parameter("cosT", [128, T], BF16, isOutput=False)
    sinT = nc.declare_dram_parameter("sinT", [128, T], F32R, isOutput=False)
    maskT = nc.declare_dram_parameter("maskT", [128, 128], BF16, isOutput=False)
    identT = nc.declare_dram_parameter("identT", [128, 128], BF16, isOutput=False)
    rotT = nc.declare_dram_parameter("rotT", [128, 128], BF16, isOutput=False)
    out = nc.declare_dram_parameter("out", [T, C], F32, isOutput=True)

    with nc.allow_low_precision("bf16 attention pipeline"), \
         tile.TileContext(nc) as tc, ExitStack() as octx:
        pool = lambda *a, **kw: octx.enter_context(tc.tile_pool(*a, **kw))
        consts = pool(name="consts", bufs=1)
        v_pool = pool(name="v", bufs=1)
        qkt_pool = pool(name="qkt", bufs=1)
        ao_pool = pool(name="ao", bufs=1)
        p_pool = pool(name="pb", bufs=2)
        avn_pool = pool(name="avnp", bufs=2)
        rec_pool = pool(name="recp", bufs=4)
        wo_pool = pool(name="wop", bufs=1)
        xt_pool = pool(name="xtp", bufs=1)
        wqk_pool = pool(name="wqkp", bufs=1)
        rtab_pool = pool(name="ropetab", bufs=1)
        rtmp_pool = pool(name="ropetmp", bufs=2)
        out_pool = pool(name="outsb", bufs=3)
        # PSUM: 3x [128,1024] scores (6 banks) + 2 shared work banks that
        # cycle projection drains, attn@V burst accumulators, transposes and
        # output-projection tiles (every tile's accesses are emitted
        # contiguously, so slot reuse never deadlocks)
        sc_ps = pool(name="scps", bufs=3, space="PSUM")
        wk_ps = pool(name="wkps", bufs=2, space="PSUM")

        mask_t = consts.tile([128, 128], BF16, tag="mask")
        ident_t = consts.tile([128, 128], BF16, tag="ident")
        rotT_t = consts.tile([128, 128], BF16, tag="rotT")

        vext_t = v_pool.tile([128, TTL, HL, Dh + 1], BF16, tag="vext", name="vext")
        vext = [vext_t[:, t_] for t_ in range(TTL)]
        # qkt[mt][half]: mt 0=Q heads01, 1=K heads01, 2=Q heads23, 3=K heads23
        qkt = [[qkt_pool.tile([128, HT], BF16, tag=f"qkt{m}_{hf}", name=f"qkt{m}_{hf}")
                for hf in range(2)] for m in range(4)]
        # ao[pair]: [128 ch (2 heads x 64), T] attention output, transposed
        ao = [ao_pool.tile([128, T], BF16, tag=f"ao{i}", name=f"ao{i}") for i in range(2)]
        wo_t = [wo_pool.tile([128, C], BF16, tag=f"wo{i}", name=f"wo{i}")
                for i in range(2)]
        wqkv_t = [wqk_pool.tile([128, 512 + HL * Dh], BF16, tag=f"wqkv{k}", name=f"wqkv{k}")
                  for k in range(CK)]
        wqk_t = [w[:, 0:512] for w in wqkv_t]
        wv_t = [w[:, 512:512 + HL * Dh] for w in wqkv_t]
        xt_t = [xt_pool.tile([128, T], BF16, tag=f"xt{k}", name=f"xt{k}")
                for k in range(CK)]
        cos_t = rtab_pool.tile([128, T], BF16, tag="cos")
        sin_t = rtab_pool.tile([128, T], F32R, tag="sin")

        state = {"avn": None}

        # ---- input DMA -------------------------------------------------
        # every DMA pays ~625ns on the shared HWDGE descriptor generator, so
        # favor few, full-width transfers, ordered by first consumption.
        for k in range(CK):
            nc.sync.dma_start(wqkv_t[k][:], wqkv[128 * k:128 * (k + 1), :])
            nc.sync.dma_start(xt_t[k][:, 0:HT], xt[128 * k:128 * (k + 1), 0:HT])
        nc.sync.dma_start(cos_t[:], cosT[:])
        nc.sync.dma_start(mask_t[:], maskT[:])
        nc.sync.dma_start(rotT_t[:], rotT[:])
        nc.sync.dma_start(ident_t[:], identT[:])
        nc.sync.dma_start(sin_t[:], sinT[:])
        for i in range(2):
            nc.sync.dma_start(wo_t[i][:], wo[128 * i:128 * (i + 1), :])
        # the softmax-denominator ones column of V, once for all kv tiles
        nc.gpsimd.memset(vext_t[:, :, :, Dh:Dh + 1], 1.0)

        def xt_dma(hf):
            for k in range(CK):
                nc.sync.dma_start(xt_t[k][:, HT * hf:HT * (hf + 1)],
                                  xt[128 * k:128 * (k + 1), HT * hf:HT * (hf + 1)])

        # ---- projections + RoPE ----------------------------------------
        rope_pending = []

        def emit_rope(m, n):
            """rotate-half via a PE permutation matmul, then the cos/sin
            elementwise combine. Emitted one projection group late so the
            PSUM->SBUF drain has completed."""
            dst = qkt[m][n // 2]
            src = dst[:, 512 * (n % 2):512 * (n % 2 + 1)]
            rps = sc_ps.tile([128, 512], F32, tag="sc", name="rps")
            nc.tensor.matmul(rps[:], rotT_t[:], src, start=True, stop=True)
            rot = rtmp_pool.tile([128, 512], BF16, tag="rot", name="rot")
            nc.vector.tensor_mul(rot[:], rps[:].bitcast(F32R),
                                 sin_t[:, 512 * n:512 * (n + 1)])
            nc.gpsimd.tensor_mul(src, src, cos_t[:, 512 * n:512 * (n + 1)])
            nc.vector.tensor_add(src, src, rot[:])

        def flush_rope():
            while rope_pending:
                emit_rope(*rope_pending.pop(0))

        def proj_group(m, n, eng="act"):
            pp = wk_ps.tile([128, 512], F32, tag="pp", name="pp")
            for k in range(CK):
                nc.tensor.matmul(pp[:], wqk_t[k][:, 128 * m:128 * (m + 1)],
                                 xt_t[k][:, 512 * n:512 * (n + 1)],
                                 start=(k == 0), stop=(k == CK - 1))
            dst = qkt[m][n // 2]
            dsl = dst[:, 512 * (n % 2):512 * (n % 2 + 1)]
            if eng == "act":
                nc.scalar.copy(dsl, pp[:])
            else:
                nc.vector.tensor_copy(dsl, pp[:])
            pending = rope_pending[:]
            rope_pending.clear()
            rope_pending.append((m, n))
            for pmn in pending:
                emit_rope(*pmn)

        def vproj_tile(t_, eng="act"):
            flush_rope()
            vp = wk_ps.tile([128, HL * Dh], F32, tag="pp", name="vp")
            for k in range(CK):
                nc.tensor.matmul(vp[:], xt_t[k][:, 128 * t_:128 * (t_ + 1)], wv_t[k][:],
                                 start=(k == 0), stop=(k == CK - 1))
            src = vp[:].rearrange("p (h d) -> p h d", h=HL)
            if eng == "act":
                nc.scalar.copy(vext[t_][:, :, 0:Dh], src)
            else:
                nc.vector.tensor_copy(vext[t_][:, :, 0:Dh], src)

        # ---- attention ---------------------------------------------------
        def attn_unit_gen(h, half, fillers, per_qt_sink=None):
            """scores^T/exp/mask + [q,ch]-oriented attn@V for head h, query
            half `half`, as a generator yielding once per kv strip (so units
            can be interleaved). `fillers` is a MUTABLE list; one closure is
            popped per strip to keep PE fed while the softmax pipeline runs,
            and callers may append more mid-flight. `per_qt_sink(qt)` (if
            set) is called right after q-tile qt is drained+transposed."""
            hp, hl = h // 2, h % 2
            qrmt, krmt = (0, 1) if h < 2 else (2, 3)
            pr = 64 * hl
            q_lo = HT * half
            qt0 = 8 * half
            n_strips = 8 if half == 0 else 16
            per_qt = per_qt_sink is not None
            strips = {}

            if hl == 0:
                avn = avn_pool.tile([128, 8, 128], BF16, tag="avn", name="avn")
                state[f"avn{hp}_{half}"] = avn
            else:
                avn = state[f"avn{hp}_{half}"]

            def transpose_qt(lqt):
                """[128 q, 128 ch] -> ao[hp][:, qcols] via PE transpose."""
                tt = wk_ps.tile([128, 128], BF16, tag="pp", name="tt")
                nc.tensor.transpose(tt[:], avn[:, lqt, :], ident_t[:])
                qtg = qt0 + lqt
                nc.vector.tensor_copy(ao[hp][:, 128 * qtg:128 * (qtg + 1)], tt[:])

            def emit_burst(qt):
                lqt = qt - qt0
                av = wk_ps.tile([128, Dh + 1], F32, tag="pp", name="av")
                for m2 in range(qt + 1):
                    p_, cs_ = strips[m2]
                    lq = 128 * qt - cs_
                    nc.tensor.matmul(av[:], p_[:, lq:lq + 128], vext[m2][:, h, :],
                                     start=(m2 == 0), stop=(m2 == qt))
                rec = rec_pool.tile([128, 1], F32, tag="rec", name="rec")
                nc.vector.reciprocal(rec[:], av[:, Dh:Dh + 1])
                # normalize out of PSUM into avn (DVE: gpsimd cannot touch PSUM)
                nc.vector.tensor_scalar_mul(
                    avn[:, lqt, pr:pr + 64], av[:, 0:Dh], rec[:])
                if per_qt:
                    transpose_qt(lqt)
                    per_qt_sink(qt)

            pending = None
            for m in range(n_strips):
                cs = max(q_lo, 128 * m)
                W = q_lo + HT - cs
                kr_t = qkt[krmt][m // 8]
                kc = 128 * m - HT * (m // 8)
                sc = sc_ps.tile([128, W], F32, tag="sc", name="sc")
                j = 0
                while 512 * j < W:
                    n = min(512, W - 512 * j)
                    qc = (cs - q_lo) + 512 * j
                    nc.tensor.matmul(
                        sc[:, 512 * j:512 * j + n],
                        kr_t[pr:pr + 64, kc:kc + 128],
                        qkt[qrmt][half][pr:pr + 64, qc:qc + n],
                        start=True, stop=True)
                    j += 1
                # strips of the second half overlap three units in flight
                p = p_pool.tile([128, W], BF16, tag=f"p{m}", name=f"p{m}",
                                bufs=3 if m < 8 else 2)
                nc.scalar.activation(p[:], sc[:, 0:W], AF.Exp, scale=0.125)
                if cs == 128 * m:
                    # gpsimd: all-SBUF bf16, keeps DVE free for PSUM drains
                    nc.gpsimd.tensor_mul(p[:, 0:128], p[:, 0:128], mask_t[:])
                strips[m] = (p, cs)
                if pending is not None:
                    emit_burst(pending)
                    pending = None
                if m >= qt0:
                    pending = m
                if m >= 1 and fillers:
                    fillers.pop(0)()
                yield
            if pending is not None:
                emit_burst(pending)
            if hl == 1 and not per_qt:
                for lqt in range(8):
                    transpose_qt(lqt)
            while fillers:
                fillers.pop(0)()

        def drive(gen):
            try:
                next(gen)
                return True
            except StopIteration:
                return False

        def attn_unit(h, half, fillers=(), per_qt_sink=None, guest=None):
            """run a unit to completion, advancing `guest` one strip per own
            strip (interleaves a later unit's Act work into this one)."""
            for _ in attn_unit_gen(h, half, list(fillers), per_qt_sink):
                if guest is not None:
                    drive(guest)

        # ---- output projection ------------------------------------------
        osb_map = {}

        def outproj_chunk(t_, n, tail=False):
            if t_ not in osb_map:
                osb_map[t_] = out_pool.tile([128, C], F32, tag="osb", name="osb")
            osb = osb_map[t_]
            op = wk_ps.tile([128, 512], F32, tag="pp", name="op")
            nc.tensor.matmul(op[:],
                             ao[0][:, 128 * t_:128 * (t_ + 1)],
                             wo_t[0][:, 512 * n:512 * (n + 1)],
                             start=True, stop=False)
            nc.tensor.matmul(op[:],
                             ao[1][:, 128 * t_:128 * (t_ + 1)],
                             wo_t[1][:, 512 * n:512 * (n + 1)],
                             start=False, stop=True)
            if tail and n == 1:
                # Act is idle in the drain tail; split engines + chunked DMA
                # to shorten the critical path
                nc.scalar.copy(osb[:, 512 * n:512 * (n + 1)], op[:])
            else:
                nc.vector.tensor_copy(osb[:, 512 * n:512 * (n + 1)], op[:])
            if tail:
                nc.sync.dma_start(out[128 * t_:128 * (t_ + 1), 512 * n:512 * (n + 1)],
                                  osb[:, 512 * n:512 * (n + 1)])
            elif n == 1:
                nc.sync.dma_start(out[128 * t_:128 * (t_ + 1), :], osb[:])
            if n == 1:
                del osb_map[t_]

        def outproj_tile(t_, tail=False):
            outproj_chunk(t_, 0, tail)
            outproj_chunk(t_, 1, tail)

        def pg(m, n, eng="act"):
            return lambda: proj_group(m, n, eng)

        def vt(t_, eng="act"):
            return lambda: vproj_tile(t_, eng)

        def oc(t_, n):
            return lambda: outproj_chunk(t_, n)

        # ---- schedule ----------------------------------------------------
        # phase 1: heads01 q/k for both halves + V for the first query half,
        # woven into the half-0 attention units (PSUM drains on Act, which
        # has slack here). The first 8 kv strips of heads 0/1 of the second
        # query half ride along as guests: their exp runs in phase-1 Act
        # slack while their PE-heavy tails stay in phase 2.
        proj_group(0, 0)
        proj_group(1, 0)
        proj_group(0, 1)
        proj_group(1, 1)
        for t_ in range(0, 4):
            vproj_tile(t_)
        attn_unit(0, 0, [vt(4), vt(5), vt(6), vt(7), pg(2, 0), pg(2, 1)])
        xt_dma(1)
        attn_unit(1, 0, [pg(3, 0), pg(3, 1), pg(0, 2), pg(1, 2), pg(0, 3), pg(1, 3)])
        f01 = [vt(8, "dve"), vt(9, "dve"), vt(12, "dve"), vt(13, "dve"),
               vt(14, "dve"), vt(15, "dve"), pg(2, 2, "dve"), pg(2, 3, "dve")]
        f11 = [vt(10, "dve"), vt(11, "dve")]
        g01 = attn_unit_gen(0, 1, f01)
        g11 = attn_unit_gen(1, 1, f11)
        attn_unit(2, 0, [flush_rope], guest=g01)
        attn_unit(3, 0, [], guest=g11)
        flush_rope()

        # phase 2: Act is saturated by exp; the deferred q/k projections and
        # the output projection keep PE fed between strips (drains on DVE).
        f01 += [pg(3, 2, "dve"), pg(3, 3, "dve"), flush_rope,
                oc(0, 0), oc(0, 1), oc(1, 0)]
        f11 += [oc(1, 1), oc(2, 0), oc(2, 1), oc(3, 0), oc(3, 1)]
        alive = True
        while alive:
            alive = drive(g01)
            alive = drive(g11) or alive
        attn_unit(2, 1, [oc(4, 0), oc(4, 1), oc(5, 0), oc(5, 1),
                         oc(6, 0), oc(6, 1), oc(7, 0), oc(7, 1)])
        attn_unit(3, 1, per_qt_sink=lambda qt: outproj_tile(qt, tail=(qt >= 14)))

    nc.finalize()
    return nc


_NC = None


def _get_nc():
    global _NC
    if _NC is None:
        _NC = build_nc()
    return _NC


def _host_tables():
    import ml_dtypes
    bf16 = ml_dtypes.bfloat16
    inv_freq = 1.0 / (10000.0 ** (np.arange(0, Dh, 2, dtype=np.float32) / Dh))  # [32]
    t = np.arange(T, dtype=np.float32)
    freqs = t[:, None] * inv_freq[None, :]                  # [T, 32]
    emb = np.concatenate([freqs, freqs], axis=-1)           # [T, 64]
    cos = np.cos(emb).T.astype(np.float32)                  # [64, T]
    sin = np.sin(emb).T.astype(np.float32)                  # [64, T]
    sin_signed = sin.copy()
    sin_signed[0:32, :] *= -1.0                             # rotate_half sign fold
    cosT = np.concatenate([cos, cos], axis=0).astype(bf16)  # [128, T] two head-halves
    sinT = np.ascontiguousarray(np.concatenate([sin_signed, sin_signed], axis=0))
    maskT = np.triu(np.ones((128, 128), np.float32)).astype(bf16)  # keep where k <= q
    identT = np.eye(128, dtype=np.float32).astype(bf16)
    sigma = np.empty(64, np.int64)
    sigma[0:32] = 2 * np.arange(32) + 1
    sigma[32:64] = 2 * np.arange(32)
    R = np.zeros((128, 128), np.float32)
    for hh in range(2):
        for d in range(64):
            R[64 * hh + d, 64 * hh + sigma[d]] = 1.0
    rotT = np.ascontiguousarray(R.T).astype(bf16)
    return cosT, sinT, maskT, identT, rotT


def kernel(x, w_qkv, w_out):
    import ml_dtypes
    bf16 = ml_dtypes.bfloat16
    x = np.asarray(x, dtype=np.float32)
    w_qkv = np.asarray(w_qkv, dtype=np.float32)
    w_out = np.asarray(w_out, dtype=np.float32)
    nc = _get_nc()
    cosT, sinT, maskT, identT, rotT = _host_tables()

    in_maps = []
    for core in range(N_CORES):
        b = core // 4
        g = core % 4
        heads = [4 * g + l for l in range(HL)]
        qcols = [w_qkv[:, 64 * h:64 * (h + 1)] for h in heads]
        kcols = [w_qkv[:, C + 64 * h:C + 64 * (h + 1)] for h in heads]
        vcols = [w_qkv[:, 2 * C + 64 * h:2 * C + 64 * (h + 1)] for h in heads]
        # m-tiles: Q01 | K01 | Q23 | K23
        wqkv_loc = np.concatenate(
            [qcols[0], qcols[1], kcols[0], kcols[1], qcols[2], qcols[3], kcols[2], kcols[3]]
            + vcols, axis=1).astype(bf16)                    # [C, 768]
        wo_loc = np.concatenate([w_out[64 * h:64 * (h + 1), :] for h in heads],
                                axis=0).astype(bf16)
        in_maps.append({
            "xt": np.ascontiguousarray(x[b].T).astype(bf16),  # [C, T]
            "wqkv": wqkv_loc,
            "wo": wo_loc,
            "cosT": cosT, "sinT": sinT, "maskT": maskT,
            "identT": identT, "rotT": rotT,
        })

    res = run_bass_kernel_spmd(nc, in_maps, core_ids=list(range(N_CORES)))
    out_arr = np.zeros((B, T, C), np.float32)
    for core in range(N_CORES):
        out_arr[core // 4] += res.results[core]["out"]
    return out_arr


# revision 26
# speedup vs baseline: 1.0271x; 1.0032x over previous
"""Multi-head self-attention (RoPE, causal) Trainium2 kernel, 8-way sharded.

Sharding: data-parallel over batch (B=2) x tensor-parallel over head groups
(16 heads -> 4 groups of 4). Core c handles batch c//4, heads 4*(c%4)..+4.
Each core computes q/k/v projections for its heads, RoPE, causal-softmax
attention, and a Megatron-style row-parallel partial of the output
projection; the host sums the 4 partials per batch.

Device dataflow (all matmul operands bf16, accumulation f32 in PSUM):
- scores are computed transposed (scores^T[kpos, q]) per 128-row kv strip,
  exp'd in one Activation op per strip into a bf16 p tile that persists for
  the head-half; causal mask is a bf16 multiply on the diagonal block only.
- attn@V runs with queries on PSUM partitions: per q-tile one contiguous
  burst of [128q x 65] matmuls accumulates p^T V over the kv strips (the
  65th V column is ones so the softmax denominator rides along; PSUM allows
  one pending accumulation group per 2KB bank, hence the burst form). This
  halves PE column count vs. streaming q on the free axis, and
  normalization becomes a native per-partition tensor_scalar multiply.
- per-q-tile PE transposes restore the [channels, q] layout the output
  projection needs as its stationary operand.
- RoPE: rotate_half is a PE permutation matmul; the sign lives in the sin
  table; the elementwise combine is split across DVE/gpsimd.
- work is phase-balanced against the Activation engine (exp is ~60us and
  binds the second query half): V projections for the second half and the
  tail head-pair q/k projections are deferred into the second half as PE
  filler, woven between attention strips.
"""
import sys
for _p in ("/opt/trn_rl_repo",):
    if _p not in sys.path:
        sys.path.insert(0, _p)

import numpy as np
from contextlib import ExitStack

import concourse.bacc as bacc
import concourse.mybir as mybir
import concourse.tile as tile
from concourse.bass_utils import run_bass_kernel_spmd

F32 = mybir.dt.float32
F32R = mybir.dt.float32r
BF16 = mybir.dt.bfloat16
AF = mybir.ActivationFunctionType

B, T, C = 2, 2048, 1024
H, Dh = 16, 64
HL = 4                      # heads per core
CK = C // 128               # 8 contraction k-tiles for projections
TTL = T // 128              # 16 T-tiles / kv k-tiles
HT = T // 2                 # 1024, the attention q-half width
N_CORES = 8


def build_nc():
    nc = bacc.Bacc("TRN2", target_bir_lowering=False, debug=False, num_devices=N_CORES)

    xt = nc.declare_dram_parameter("xt", [C, T], BF16, isOutput=False)
    wqkv = nc.declare_dram_parameter("wqkv", [C, 4 * 128 + HL * Dh], BF16, isOutput=False)
    wo = nc.declare_dram_parameter("wo", [HL * Dh, C], BF16, isOutput=False)
    cosT = nc.declare_dram_parameter("cosT", [128, T], BF16, isOutput=False)
    sinT = nc.declare_dram_parameter("sinT", [128, T], F32R, isOutput=False)
    maskT = nc.declare_dram_parameter("maskT", [128, 128], BF16, isOutput=False)
    identT = nc.declare_dram_parameter("identT", [128, 128], BF16, isOutput=False)
    rotT = nc.declare_dram_parameter("rotT", [128, 128], BF16, isOutput=False)
    out = nc.declare_dram_parameter("out", [T, C], F32, isOutput=True)

    with nc.allow_low_precision("bf16 attention pipeline"), \
         tile.TileContext(nc) as tc, ExitStack() as octx:
        pool = lambda *a, **kw: octx.enter_context(tc.tile_pool(*a, **kw))
        consts = pool(name="consts", bufs=1)
        v_pool = pool(name="v", bufs=1)
        qkt_pool = pool(name="qkt", bufs=1)
        ao_pool = pool(name="ao", bufs=1)
        p_pool = pool(name="pb", bufs=2)
        avn_pool = pool(name="avnp", bufs=2)
        rec_pool = pool(name="recp", bufs=4)
        wo_pool = pool(name="wop", bufs=1)
        xt_pool = pool(name="xtp", bufs=1)
        wqk_pool = pool(name="wqkp", bufs=1)
        rtab_pool = pool(name="ropetab", bufs=1)
        rtmp_pool = pool(name="ropetmp", bufs=2)
        out_pool = pool(name="outsb", bufs=3)
        # PSUM: 3x [128,1024] scores (6 banks) + 2 shared work banks that
        # cycle projection drains, attn@V burst accumulators, transposes and
        # output-projection tiles (every tile's accesses are emitted
        # contiguously, so slot reuse never deadlocks)
        sc_ps = pool(name="scps", bufs=3, space="PSUM")
        wk_ps = pool(name="wkps", bufs=2, space="PSUM")

        mask_t = consts.tile([128, 128], BF16, tag="mask")
        ident_t = consts.tile([128, 128], BF16, tag="ident")
        rotT_t = consts.tile([128, 128], BF16, tag="rotT")

        vext_t = v_pool.tile([128, TTL, HL, Dh + 1], BF16, tag="vext", name="vext")
        vext = [vext_t[:, t_] for t_ in range(TTL)]
        # qkt[mt][half]: mt 0=Q heads01, 1=K heads01, 2=Q heads23, 3=K heads23
        qkt = [[qkt_pool.tile([128, HT], BF16, tag=f"qkt{m}_{hf}", name=f"qkt{m}_{hf}")
                for hf in range(2)] for m in range(4)]
        # ao[pair]: [128 ch (2 heads x 64), T] attention output, transposed
        ao = [ao_pool.tile([128, T], BF16, tag=f"ao{i}", name=f"ao{i}") for i in range(2)]
        wo_t = [wo_pool.tile([128, C], BF16, tag=f"wo{i}", name=f"wo{i}")
                for i in range(2)]
        wqkv_t = [wqk_pool.tile([128, 512 + HL * Dh], BF16, tag=f"wqkv{k}", name=f"wqkv{k}")
                  for k in range(CK)]
        wqk_t = [w[:, 0:512] for w in wqkv_t]
        wv_t = [w[:, 512:512 + HL * Dh] for w in wqkv_t]
        xt_t = [xt_pool.tile([128, T], BF16, tag=f"xt{k}", name=f"xt{k}")
                for k in range(CK)]
        cos_t = rtab_pool.tile([128, T], BF16, tag="cos")
        sin_t = rtab_pool.tile([128, T], F32R, tag="sin")

        state = {"avn": None}

        # ---- input DMA -------------------------------------------------
        # every DMA pays ~625ns on the shared HWDGE descriptor generator and
        # the transfer bus is ~360GB/s shared, so favor few transfers,
        # ordered exactly by first consumption.
        for k in range(CK):
            nc.sync.dma_start(xt_t[k][:, 0:HT], xt[128 * k:128 * (k + 1), 0:HT])
            nc.sync.dma_start(wqkv_t[k][:, 0:512], wqkv[128 * k:128 * (k + 1), 0:512])
        nc.sync.dma_start(rotT_t[:], rotT[:])
        nc.sync.dma_start(cos_t[:], cosT[:])
        for k in range(CK):   # V weight columns, for the prologue vprojs
            nc.sync.dma_start(wqkv_t[k][:, 512:768], wqkv[128 * k:128 * (k + 1), 512:768])
        nc.sync.dma_start(sin_t[:, 0:HT], sinT[:, 0:HT])
        nc.sync.dma_start(mask_t[:], maskT[:])
        nc.sync.dma_start(ident_t[:], identT[:])
        nc.sync.dma_start(sin_t[:, HT:T], sinT[:, HT:T])
        for i in range(2):
            nc.sync.dma_start(wo_t[i][:], wo[128 * i:128 * (i + 1), :])
        # the softmax-denominator ones column of V, once for all kv tiles
        nc.gpsimd.memset(vext_t[:, :, :, Dh:Dh + 1], 1.0)

        def xt_dma(hf):
            for k in range(CK):
                nc.sync.dma_start(xt_t[k][:, HT * hf:HT * (hf + 1)],
                                  xt[128 * k:128 * (k + 1), HT * hf:HT * (hf + 1)])

        # ---- projections + RoPE ----------------------------------------
        rope_pending = []

        def emit_rope(m, n):
            """rotate-half via a PE permutation matmul, then the cos/sin
            elementwise combine. Emitted one projection group late so the
            PSUM->SBUF drain has completed."""
            dst = qkt[m][n // 2]
            src = dst[:, 512 * (n % 2):512 * (n % 2 + 1)]
            rps = sc_ps.tile([128, 512], F32, tag="sc", name="rps")
            nc.tensor.matmul(rps[:], rotT_t[:], src, start=True, stop=True)
            rot = rtmp_pool.tile([128, 512], BF16, tag="rot", name="rot")
            nc.vector.tensor_mul(rot[:], rps[:].bitcast(F32R),
                                 sin_t[:, 512 * n:512 * (n + 1)])
            nc.gpsimd.tensor_mul(src, src, cos_t[:, 512 * n:512 * (n + 1)])
            nc.vector.tensor_add(src, src, rot[:])

        def flush_rope():
            while rope_pending:
                emit_rope(*rope_pending.pop(0))

        def proj_group(m, n, eng="act"):
            pp = wk_ps.tile([128, 512], F32, tag="pp", name="pp")
            for k in range(CK):
                nc.tensor.matmul(pp[:], wqk_t[k][:, 128 * m:128 * (m + 1)],
                                 xt_t[k][:, 512 * n:512 * (n + 1)],
                                 start=(k == 0), stop=(k == CK - 1))
            dst = qkt[m][n // 2]
            dsl = dst[:, 512 * (n % 2):512 * (n % 2 + 1)]
            if eng == "act":
                nc.scalar.copy(dsl, pp[:])
            else:
                nc.vector.tensor_copy(dsl, pp[:])
            pending = rope_pending[:]
            rope_pending.clear()
            rope_pending.append((m, n))
            for pmn in pending:
                emit_rope(*pmn)

        def vproj_tile(t_, eng="act", flush=True):
            if flush:
                flush_rope()
            vp = wk_ps.tile([128, HL * Dh], F32, tag="pp", name="vp")
            for k in range(CK):
                nc.tensor.matmul(vp[:], xt_t[k][:, 128 * t_:128 * (t_ + 1)], wv_t[k][:],
                                 start=(k == 0), stop=(k == CK - 1))
            src = vp[:].rearrange("p (h d) -> p h d", h=HL)
            if eng == "act":
                nc.scalar.copy(vext[t_][:, :, 0:Dh], src)
            else:
                nc.vector.tensor_copy(vext[t_][:, :, 0:Dh], src)

        # ---- attention ---------------------------------------------------
        def attn_unit_gen(h, half, fillers, per_qt_sink=None):
            """scores^T/exp/mask + [q,ch]-oriented attn@V for head h, query
            half `half`, as a generator yielding once per kv strip (so units
            can be interleaved). `fillers` is a MUTABLE list; one closure is
            popped per strip to keep PE fed while the softmax pipeline runs,
            and callers may append more mid-flight. `per_qt_sink(qt)` (if
            set) is called right after q-tile qt is drained+transposed."""
            hp, hl = h // 2, h % 2
            qrmt, krmt = (0, 1) if h < 2 else (2, 3)
            pr = 64 * hl
            q_lo = HT * half
            qt0 = 8 * half
            n_strips = 8 if half == 0 else 16
            per_qt = per_qt_sink is not None
            strips = {}

            if hl == 0:
                avn = avn_pool.tile([128, 8, 128], BF16, tag="avn", name="avn")
                state[f"avn{hp}_{half}"] = avn
            else:
                avn = state[f"avn{hp}_{half}"]

            def transpose_qt(lqt, act=False):
                """[128 q, 128 ch] -> ao[hp][:, qcols] via PE transpose."""
                tt = wk_ps.tile([128, 128], BF16, tag="pp", name="tt")
                nc.tensor.transpose(tt[:], avn[:, lqt, :], ident_t[:])
                qtg = qt0 + lqt
                dst = ao[hp][:, 128 * qtg:128 * (qtg + 1)]
                if act:
                    nc.scalar.copy(dst, tt[:])
                else:
                    nc.vector.tensor_copy(dst, tt[:])

            def emit_burst(qt):
                lqt = qt - qt0
                av = wk_ps.tile([128, Dh + 1], F32, tag="pp", name="av")
                for m2 in range(qt + 1):
                    p_, cs_ = strips[m2]
                    lq = 128 * qt - cs_
                    nc.tensor.matmul(av[:], p_[:, lq:lq + 128], vext[m2][:, h, :],
                                     start=(m2 == 0), stop=(m2 == qt))
                rec = rec_pool.tile([128, 1], F32, tag="rec", name="rec")
                nc.vector.reciprocal(rec[:], av[:, Dh:Dh + 1])
                # normalize out of PSUM into avn; Act takes over in the drain
                # tail where exp is finished (gpsimd cannot touch PSUM)
                in_tail = per_qt and qt >= 13
                if in_tail:
                    nc.scalar.activation(avn[:, lqt, pr:pr + 64], av[:, 0:Dh],
                                         AF.Copy, scale=rec[:])
                else:
                    nc.vector.tensor_scalar_mul(
                        avn[:, lqt, pr:pr + 64], av[:, 0:Dh], rec[:])
                if per_qt:
                    transpose_qt(lqt, act=in_tail)
                    per_qt_sink(qt)

            pending = None
            for m in range(n_strips):
                cs = max(q_lo, 128 * m)
                W = q_lo + HT - cs
                kr_t = qkt[krmt][m // 8]
                kc = 128 * m - HT * (m // 8)
                sc = sc_ps.tile([128, W], F32, tag="sc", name="sc")
                j = 0
                while 512 * j < W:
                    n = min(512, W - 512 * j)
                    qc = (cs - q_lo) + 512 * j
                    nc.tensor.matmul(
                        sc[:, 512 * j:512 * j + n],
                        kr_t[pr:pr + 64, kc:kc + 128],
                        qkt[qrmt][half][pr:pr + 64, qc:qc + n],
                        start=True, stop=True)
                    j += 1
                # strips of the second half overlap three units in flight
                p = p_pool.tile([128, W], BF16, tag=f"p{m}", name=f"p{m}",
                                bufs=3 if m < 8 else 2)
                nc.scalar.activation(p[:], sc[:, 0:W], AF.Exp, scale=0.125)
                if cs == 128 * m:
                    # gpsimd: all-SBUF bf16, keeps DVE free for PSUM drains
                    nc.gpsimd.tensor_mul(p[:, 0:128], p[:, 0:128], mask_t[:])
                strips[m] = (p, cs)
                if pending is not None:
                    emit_burst(pending)
                    pending = None
                if m >= qt0:
                    pending = m
                if m >= 1 and fillers:
                    fillers.pop(0)()
                yield
            if pending is not None:
                emit_burst(pending)
            if hl == 1 and not per_qt:
                for lqt in range(8):
                    transpose_qt(lqt)
            while fillers:
                fillers.pop(0)()

        def drive(gen):
            try:
                next(gen)
                return True
            except StopIteration:
                return False

        def attn_unit(h, half, fillers=(), per_qt_sink=None, guest=None):
            """run a unit to completion, advancing `guest` one strip per own
            strip (interleaves a later unit's Act work into this one)."""
            for _ in attn_unit_gen(h, half, list(fillers), per_qt_sink):
                if guest is not None:
                    drive(guest)

        # ---- output projection ------------------------------------------
        osb_map = {}

        def outproj_chunk(t_, n, tail=False):
            if t_ not in osb_map:
                osb_map[t_] = out_pool.tile([128, C], F32, tag="osb", name="osb")
            osb = osb_map[t_]
            op = wk_ps.tile([128, 512], F32, tag="pp", name="op")
            nc.tensor.matmul(op[:],
                             ao[0][:, 128 * t_:128 * (t_ + 1)],
                             wo_t[0][:, 512 * n:512 * (n + 1)],
                             start=True, stop=False)
            nc.tensor.matmul(op[:],
                             ao[1][:, 128 * t_:128 * (t_ + 1)],
                             wo_t[1][:, 512 * n:512 * (n + 1)],
                             start=False, stop=True)
            if tail and n == 1:
                # Act is idle in the drain tail; split engines + chunked DMA
                # to shorten the critical path
                nc.scalar.copy(osb[:, 512 * n:512 * (n + 1)], op[:])
            else:
                nc.vector.tensor_copy(osb[:, 512 * n:512 * (n + 1)], op[:])
            if tail:
                nc.sync.dma_start(out[128 * t_:128 * (t_ + 1), 512 * n:512 * (n + 1)],
                                  osb[:, 512 * n:512 * (n + 1)])
            elif n == 1:
                nc.sync.dma_start(out[128 * t_:128 * (t_ + 1), :], osb[:])
            if n == 1:
                del osb_map[t_]

        def outproj_tile(t_, tail=False):
            outproj_chunk(t_, 0, tail)
            outproj_chunk(t_, 1, tail)

        def pg(m, n, eng="act"):
            return lambda: proj_group(m, n, eng)

        def vt(t_, eng="act"):
            return lambda: vproj_tile(t_, eng)

        def oc(t_, n):
            return lambda: outproj_chunk(t_, n)

        # ---- schedule ----------------------------------------------------
        # prologue: the first four projection groups run k-interleaved so PE
        # consumes each (xt[k], wqkv[k]) DMA pair the moment it lands,
        # accumulating into four concurrent PSUM regions (scores pool is
        # still free). V tiles 0-5 follow while tables stream in.
        pro = [(0, 0), (1, 0), (0, 1), (1, 1)]
        pps = [(sc_ps if i < 3 else wk_ps).tile([128, 512], F32,
                                                tag="sc" if i < 3 else "pp",
                                                name=f"pp{i}")
               for i in range(4)]
        for k in range(CK):
            for (m, n), pp in zip(pro, pps):
                nc.tensor.matmul(pp[:], wqk_t[k][:, 128 * m:128 * (m + 1)],
                                 xt_t[k][:, 512 * n:512 * (n + 1)],
                                 start=(k == 0), stop=(k == CK - 1))

        def drain_pro(i):
            m, n = pro[i]
            nc.scalar.copy(qkt[m][n // 2][:, 512 * (n % 2):512 * (n % 2 + 1)],
                           pps[i][:])
            rope_pending.append((m, n))

        drain_pro(0)
        drain_pro(1)
        vproj_tile(0, flush=False)
        vproj_tile(1, flush=False)
        drain_pro(2)
        drain_pro(3)
        vproj_tile(2)   # flushes the four prologue ropes
        vproj_tile(3)
        vproj_tile(4)
        vproj_tile(5)

        # phase 1: remaining projections woven into the half-0 attention
        # units (PSUM drains on Act, which has slack here). The first 8 kv
        # strips of heads 0/1 of the second query half ride along as guests:
        # their exp runs in phase-1 Act slack while their PE-heavy tails
        # stay in phase 2.
        attn_unit(0, 0, [vt(6), vt(7), pg(2, 0), pg(2, 1)])
        xt_dma(1)
        attn_unit(1, 0, [pg(3, 0), pg(3, 1), pg(0, 2), pg(1, 2), pg(0, 3), pg(1, 3)])
        f01 = [vt(8, "dve"), vt(9, "dve"), vt(12, "dve"), vt(13, "dve"),
               vt(14, "dve"), vt(15, "dve"), pg(2, 2, "dve"), pg(2, 3, "dve")]
        f11 = [vt(10, "dve"), vt(11, "dve"), pg(3, 2, "dve"), pg(3, 3, "dve")]
        g01 = attn_unit_gen(0, 1, f01)
        g11 = attn_unit_gen(1, 1, f11)
        attn_unit(2, 0, [flush_rope], guest=g01)
        attn_unit(3, 0, [], guest=g11)
        flush_rope()

        # phase 2: Act is saturated by exp; the deferred q/k projections and
        # the output projection keep PE fed between strips (drains on DVE).
        f01 += [flush_rope, oc(0, 0), oc(0, 1), oc(1, 0)]
        f11 += [oc(1, 1), oc(2, 0), oc(2, 1), oc(3, 0), oc(3, 1)]
        alive = True
        while alive:
            alive = drive(g01)
            alive = drive(g11) or alive
        attn_unit(2, 1, [oc(4, 0), oc(4, 1), oc(5, 0), oc(5, 1),
                         oc(6, 0), oc(6, 1), oc(7, 0), oc(7, 1)])
        attn_unit(3, 1, per_qt_sink=lambda qt: outproj_tile(qt, tail=(qt >= 12)))

    nc.finalize()
    return nc


_NC = None


def _get_nc():
    global _NC
    if _NC is None:
        _NC = build_nc()
    return _NC


def _host_tables():
    import ml_dtypes
    bf16 = ml_dtypes.bfloat16
    inv_freq = 1.0 / (10000.0 ** (np.arange(0, Dh, 2, dtype=np.float32) / Dh))  # [32]
    t = np.arange(T, dtype=np.float32)
    freqs = t[:, None] * inv_freq[None, :]                  # [T, 32]
    emb = np.concatenate([freqs, freqs], axis=-1)           # [T, 64]
    cos = np.cos(emb).T.astype(np.float32)                  # [64, T]
    sin = np.sin(emb).T.astype(np.float32)                  # [64, T]
    sin_signed = sin.copy()
    sin_signed[0:32, :] *= -1.0                             # rotate_half sign fold
    cosT = np.concatenate([cos, cos], axis=0).astype(bf16)  # [128, T] two head-halves
    sinT = np.ascontiguousarray(np.concatenate([sin_signed, sin_signed], axis=0))
    maskT = np.triu(np.ones((128, 128), np.float32)).astype(bf16)  # keep where k <= q
    identT = np.eye(128, dtype=np.float32).astype(bf16)
    sigma = np.empty(64, np.int64)
    sigma[0:32] = 2 * np.arange(32) + 1
    sigma[32:64] = 2 * np.arange(32)
    R = np.zeros((128, 128), np.float32)
    for hh in range(2):
        for d in range(64):
            R[64 * hh + d, 64 * hh + sigma[d]] = 1.0
    rotT = np.ascontiguousarray(R.T).astype(bf16)
    return cosT, sinT, maskT, identT, rotT


def kernel(x, w_qkv, w_out):
    import ml_dtypes
    bf16 = ml_dtypes.bfloat16
    x = np.asarray(x, dtype=np.float32)
    w_qkv = np.asarray(w_qkv, dtype=np.float32)
    w_out = np.asarray(w_out, dtype=np.float32)
    nc = _get_nc()
    cosT, sinT, maskT, identT, rotT = _host_tables()

    in_maps = []
    for core in range(N_CORES):
        b = core // 4
        g = core % 4
        heads = [4 * g + l for l in range(HL)]
        qcols = [w_qkv[:, 64 * h:64 * (h + 1)] for h in heads]
        kcols = [w_qkv[:, C + 64 * h:C + 64 * (h + 1)] for h in heads]
        vcols = [w_qkv[:, 2 * C + 64 * h:2 * C + 64 * (h + 1)] for h in heads]
        # m-tiles: Q01 | K01 | Q23 | K23
        wqkv_loc = np.concatenate(
            [qcols[0], qcols[1], kcols[0], kcols[1], qcols[2], qcols[3], kcols[2], kcols[3]]
            + vcols, axis=1).astype(bf16)                    # [C, 768]
        wo_loc = np.concatenate([w_out[64 * h:64 * (h + 1), :] for h in heads],
                                axis=0).astype(bf16)
        in_maps.append({
            "xt": np.ascontiguousarray(x[b].T).astype(bf16),  # [C, T]
            "wqkv": wqkv_loc,
            "wo": wo_loc,
            "cosT": cosT, "sinT": sinT, "maskT": maskT,
            "identT": identT, "rotT": rotT,
        })

    res = run_bass_kernel_spmd(nc, in_maps, core_ids=list(range(N_CORES)))
    out_arr = np.zeros((B, T, C), np.float32)
    for core in range(N_CORES):
        out_arr[core // 4] += res.results[core]["out"]
    return out_arr


# revision 30
# speedup vs baseline: 1.0953x; 1.0663x over previous
"""Multi-head self-attention (RoPE, causal) Trainium2 kernel, 8-way sharded.

Sharding: data-parallel over batch (B=2) x tensor-parallel over head groups
(16 heads -> 4 groups of 4). Core c handles batch c//4, heads 4*(c%4)..+4.
Each core computes q/k/v projections for its heads, RoPE, causal-softmax
attention, and a Megatron-style row-parallel partial of the output
projection; the host sums the 4 partials per batch.

Device dataflow (all matmul operands bf16, accumulation f32 in PSUM):
- scores are computed transposed (scores^T[kpos, q]) per 128-row kv strip,
  exp'd in one Activation op per strip into a bf16 p tile that persists for
  the head-half; causal mask is a bf16 multiply on the diagonal block only.
- attn@V runs with queries on PSUM partitions: per q-tile one contiguous
  burst of [128q x 65] matmuls accumulates p^T V over the kv strips (the
  65th V column is ones so the softmax denominator rides along; PSUM allows
  one pending accumulation group per 2KB bank, hence the burst form). This
  halves PE column count vs. streaming q on the free axis, and
  normalization becomes a native per-partition tensor_scalar multiply.
- per-q-tile PE transposes restore the [channels, q] layout the output
  projection needs as its stationary operand.
- RoPE: rotate_half is a PE permutation matmul; the sign lives in the sin
  table; the elementwise combine is split across DVE/gpsimd.
- work is phase-balanced against the Activation engine (exp is ~60us and
  binds the second query half): V projections for the second half and the
  tail head-pair q/k projections are deferred into the second half as PE
  filler, woven between attention strips.
"""
import sys
for _p in ("/opt/trn_rl_repo",):
    if _p not in sys.path:
        sys.path.insert(0, _p)

import numpy as np
from contextlib import ExitStack

import concourse.bacc as bacc
import concourse.mybir as mybir
import concourse.tile as tile
from concourse.bass_utils import run_bass_kernel_spmd

F32 = mybir.dt.float32
F32R = mybir.dt.float32r
BF16 = mybir.dt.bfloat16
AF = mybir.ActivationFunctionType

B, T, C = 2, 2048, 1024
H, Dh = 16, 64
HL = 4                      # heads per core
CK = C // 128               # 8 contraction k-tiles for projections
TTL = T // 128              # 16 T-tiles / kv k-tiles
HT = T // 2                 # 1024, the attention q-half width
N_CORES = 8


def build_nc():
    nc = bacc.Bacc("TRN2", target_bir_lowering=False, debug=False, num_devices=N_CORES)

    xt = nc.declare_dram_parameter("xt", [C, T], BF16, isOutput=False)
    wqkv = nc.declare_dram_parameter("wqkv", [C, 4 * 128 + HL * Dh], BF16, isOutput=False)
    wo = nc.declare_dram_parameter("wo", [HL * Dh, C], BF16, isOutput=False)
    cosT = nc.declare_dram_parameter("cosT", [128, T], BF16, isOutput=False)
    sinT = nc.declare_dram_parameter("sinT", [128, T], F32R, isOutput=False)
    maskT = nc.declare_dram_parameter("maskT", [128, 128], BF16, isOutput=False)
    identT = nc.declare_dram_parameter("identT", [128, 128], BF16, isOutput=False)
    rotT = nc.declare_dram_parameter("rotT", [128, 128], BF16, isOutput=False)
    out = nc.declare_dram_parameter("out", [T, C], F32, isOutput=True)

    with nc.allow_low_precision("bf16 attention pipeline"), \
         tile.TileContext(nc) as tc, ExitStack() as octx:
        pool = lambda *a, **kw: octx.enter_context(tc.tile_pool(*a, **kw))
        consts = pool(name="consts", bufs=1)
        v_pool = pool(name="v", bufs=1)
        qkt_pool = pool(name="qkt", bufs=1)
        ao_pool = pool(name="ao", bufs=1)
        p_pool = pool(name="pb", bufs=2)
        avn_pool = pool(name="avnp", bufs=2)
        rec_pool = pool(name="recp", bufs=4)
        wo_pool = pool(name="wop", bufs=1)
        xt_pool = pool(name="xtp", bufs=1)
        wqk_pool = pool(name="wqkp", bufs=1)
        rtab_pool = pool(name="ropetab", bufs=1)
        rtmp_pool = pool(name="ropetmp", bufs=2)
        out_pool = pool(name="outsb", bufs=3)
        # PSUM: 3x [128,1024] scores (6 banks) + 2 shared work banks that
        # cycle projection drains, attn@V burst accumulators, transposes and
        # output-projection tiles (every tile's accesses are emitted
        # contiguously, so slot reuse never deadlocks)
        sc_ps = pool(name="scps", bufs=3, space="PSUM")
        wk_ps = pool(name="wkps", bufs=2, space="PSUM")

        mask_t = consts.tile([128, 128], BF16, tag="mask")
        ident_t = consts.tile([128, 128], BF16, tag="ident")
        rotT_t = consts.tile([128, 128], BF16, tag="rotT")

        vext_t = v_pool.tile([128, TTL, HL, Dh + 1], BF16, tag="vext", name="vext")
        vext = [vext_t[:, t_] for t_ in range(TTL)]
        # qkt[mt][half]: mt 0=Q heads01, 1=K heads01, 2=Q heads23, 3=K heads23
        qkt = [[qkt_pool.tile([128, HT], BF16, tag=f"qkt{m}_{hf}", name=f"qkt{m}_{hf}")
                for hf in range(2)] for m in range(4)]
        # ao[pair]: [128 ch (2 heads x 64), T] attention output, transposed
        ao = [ao_pool.tile([128, T], BF16, tag=f"ao{i}", name=f"ao{i}") for i in range(2)]
        wo_t = [wo_pool.tile([128, C], BF16, tag=f"wo{i}", name=f"wo{i}")
                for i in range(2)]
        wqkv_t = [wqk_pool.tile([128, 512 + HL * Dh], BF16, tag=f"wqkv{k}", name=f"wqkv{k}")
                  for k in range(CK)]
        wqk_t = [w[:, 0:512] for w in wqkv_t]
        wv_t = [w[:, 512:512 + HL * Dh] for w in wqkv_t]
        xt_t = [xt_pool.tile([128, T], BF16, tag=f"xt{k}", name=f"xt{k}")
                for k in range(CK)]
        cos_t = rtab_pool.tile([128, T], BF16, tag="cos")
        sin_t = rtab_pool.tile([128, T], F32R, tag="sin")

        state = {"avn": None}

        # ---- input DMA -------------------------------------------------
        # every DMA pays ~625ns on the shared HWDGE descriptor generator and
        # the transfer bus is ~360GB/s shared, so favor few transfers,
        # ordered exactly by first consumption.
        for k in range(CK):
            nc.sync.dma_start(xt_t[k][:, 0:HT], xt[128 * k:128 * (k + 1), 0:HT])
            nc.sync.dma_start(wqkv_t[k][:, 0:512], wqkv[128 * k:128 * (k + 1), 0:512])
        nc.sync.dma_start(rotT_t[:], rotT[:])
        nc.sync.dma_start(cos_t[:], cosT[:])
        for k in range(CK):   # V weight columns, for the prologue vprojs
            nc.sync.dma_start(wqkv_t[k][:, 512:768], wqkv[128 * k:128 * (k + 1), 512:768])
        nc.sync.dma_start(sin_t[:, 0:HT], sinT[:, 0:HT])
        nc.sync.dma_start(mask_t[:], maskT[:])
        nc.sync.dma_start(ident_t[:], identT[:])
        nc.sync.dma_start(sin_t[:, HT:T], sinT[:, HT:T])
        for i in range(2):
            nc.sync.dma_start(wo_t[i][:], wo[128 * i:128 * (i + 1), :])
        # the softmax-denominator ones column of V, once for all kv tiles
        nc.gpsimd.memset(vext_t[:, :, :, Dh:Dh + 1], 1.0)

        def xt_dma(hf):
            for k in range(CK):
                nc.sync.dma_start(xt_t[k][:, HT * hf:HT * (hf + 1)],
                                  xt[128 * k:128 * (k + 1), HT * hf:HT * (hf + 1)])

        # ---- projections + RoPE ----------------------------------------
        rope_pending = []

        def emit_rope(m, n):
            """rotate-half via a PE permutation matmul, then the cos/sin
            elementwise combine. Emitted one projection group late so the
            PSUM->SBUF drain has completed."""
            dst = qkt[m][n // 2]
            src = dst[:, 512 * (n % 2):512 * (n % 2 + 1)]
            rps = sc_ps.tile([128, 512], F32, tag="sc", name="rps")
            nc.tensor.matmul(rps[:], rotT_t[:], src, start=True, stop=True)
            rot = rtmp_pool.tile([128, 512], BF16, tag="rot", name="rot")
            nc.vector.tensor_mul(rot[:], rps[:].bitcast(F32R),
                                 sin_t[:, 512 * n:512 * (n + 1)])
            nc.gpsimd.tensor_mul(src, src, cos_t[:, 512 * n:512 * (n + 1)])
            nc.vector.tensor_add(src, src, rot[:])

        def flush_rope():
            while rope_pending:
                emit_rope(*rope_pending.pop(0))

        def proj_group(m, n, eng="act"):
            pp = wk_ps.tile([128, 512], F32, tag="pp", name="pp")
            for k in range(CK):
                nc.tensor.matmul(pp[:], wqk_t[k][:, 128 * m:128 * (m + 1)],
                                 xt_t[k][:, 512 * n:512 * (n + 1)],
                                 start=(k == 0), stop=(k == CK - 1))
            dst = qkt[m][n // 2]
            dsl = dst[:, 512 * (n % 2):512 * (n % 2 + 1)]
            if eng == "act":
                nc.scalar.copy(dsl, pp[:])
            else:
                nc.vector.tensor_copy(dsl, pp[:])
            pending = rope_pending[:]
            rope_pending.clear()
            rope_pending.append((m, n))
            for pmn in pending:
                emit_rope(*pmn)

        def vproj_tile(t_, eng="act", flush=True):
            if flush:
                flush_rope()
            vp = wk_ps.tile([128, HL * Dh], F32, tag="pp", name="vp")
            for k in range(CK):
                nc.tensor.matmul(vp[:], xt_t[k][:, 128 * t_:128 * (t_ + 1)], wv_t[k][:],
                                 start=(k == 0), stop=(k == CK - 1))
            src = vp[:].rearrange("p (h d) -> p h d", h=HL)
            if eng == "act":
                nc.scalar.copy(vext[t_][:, :, 0:Dh], src)
            else:
                nc.vector.tensor_copy(vext[t_][:, :, 0:Dh], src)

        # ---- attention ---------------------------------------------------
        def attn_unit_gen(h, half, fillers, per_qt_sink=None):
            """scores^T/exp/mask + [q,ch]-oriented attn@V for head h, query
            half `half`, as a generator yielding once per kv strip (so units
            can be interleaved). `fillers` is a MUTABLE list; one closure is
            popped per strip to keep PE fed while the softmax pipeline runs,
            and callers may append more mid-flight. `per_qt_sink(qt)` (if
            set) is called right after q-tile qt is drained+transposed."""
            hp, hl = h // 2, h % 2
            qrmt, krmt = (0, 1) if h < 2 else (2, 3)
            pr = 64 * hl
            q_lo = HT * half
            qt0 = 8 * half
            n_strips = 8 if half == 0 else 16
            per_qt = per_qt_sink is not None
            strips = {}

            if hl == 0:
                avn = avn_pool.tile([128, 8, 128], BF16, tag="avn", name="avn")
                state[f"avn{hp}_{half}"] = avn
            else:
                avn = state[f"avn{hp}_{half}"]

            def transpose_qt(lqt, act=False):
                """[128 q, 128 ch] -> ao[hp][:, qcols] via PE transpose."""
                tt = wk_ps.tile([128, 128], BF16, tag="pp", name="tt")
                nc.tensor.transpose(tt[:], avn[:, lqt, :], ident_t[:])
                qtg = qt0 + lqt
                dst = ao[hp][:, 128 * qtg:128 * (qtg + 1)]
                if act:
                    nc.scalar.copy(dst, tt[:])
                else:
                    nc.vector.tensor_copy(dst, tt[:])

            tail_pending = []

            def flush_tail(lqt):
                """transpose + sink one strip behind the drain chain, so the
                cross-engine recip/normalize latency never blocks PE."""
                in_tail = lqt + qt0 >= 13
                transpose_qt(lqt, act=in_tail)
                per_qt_sink(qt0 + lqt)

            def emit_burst(qt):
                lqt = qt - qt0
                av = sc_ps.tile([128, Dh + 1], F32, tag="sc", name="av")
                for m2 in range(qt + 1):
                    p_, cs_ = strips[m2]
                    lq = 128 * qt - cs_
                    nc.tensor.matmul(av[:], p_[:, lq:lq + 128], vext[m2][:, h, :],
                                     start=(m2 == 0), stop=(m2 == qt))
                rec = rec_pool.tile([128, 1], F32, tag="rec", name="rec")
                nc.vector.reciprocal(rec[:], av[:, Dh:Dh + 1])
                # normalize out of PSUM into avn; Act takes over in the drain
                # tail where exp is finished (gpsimd cannot touch PSUM)
                if per_qt and qt >= 13:
                    nc.scalar.activation(avn[:, lqt, pr:pr + 64], av[:, 0:Dh],
                                         AF.Copy, scale=rec[:])
                else:
                    nc.vector.tensor_scalar_mul(
                        avn[:, lqt, pr:pr + 64], av[:, 0:Dh], rec[:])
                if per_qt:
                    tail_pending.append(lqt)
                    if len(tail_pending) >= 2:
                        flush_tail(tail_pending.pop(0))

            pending = None
            for m in range(n_strips):
                cs = max(q_lo, 128 * m)
                W = q_lo + HT - cs
                kr_t = qkt[krmt][m // 8]
                kc = 128 * m - HT * (m // 8)
                sc = sc_ps.tile([128, W], F32, tag="sc", name="sc")
                j = 0
                while 512 * j < W:
                    n = min(512, W - 512 * j)
                    qc = (cs - q_lo) + 512 * j
                    nc.tensor.matmul(
                        sc[:, 512 * j:512 * j + n],
                        kr_t[pr:pr + 64, kc:kc + 128],
                        qkt[qrmt][half][pr:pr + 64, qc:qc + n],
                        start=True, stop=True)
                    j += 1
                # strips of the second half overlap three units in flight
                p = p_pool.tile([128, W], BF16, tag=f"p{m}", name=f"p{m}",
                                bufs=3 if m < 8 else 2)
                nc.scalar.activation(p[:], sc[:, 0:W], AF.Exp, scale=0.125)
                if cs == 128 * m:
                    # gpsimd: all-SBUF bf16, keeps DVE free for PSUM drains
                    nc.gpsimd.tensor_mul(p[:, 0:128], p[:, 0:128], mask_t[:])
                strips[m] = (p, cs)
                if pending is not None:
                    emit_burst(pending)
                    pending = None
                if m >= qt0:
                    pending = m
                if m >= 1 and fillers:
                    fillers.pop(0)()
                yield
            if pending is not None:
                emit_burst(pending)
            while tail_pending:
                flush_tail(tail_pending.pop(0))
            if hl == 1 and not per_qt:
                for lqt in range(8):
                    transpose_qt(lqt)
            while fillers:
                fillers.pop(0)()

        def drive(gen):
            try:
                next(gen)
                return True
            except StopIteration:
                return False

        def attn_unit(h, half, fillers=(), per_qt_sink=None, guest=None):
            """run a unit to completion, advancing `guest` one strip per own
            strip (interleaves a later unit's Act work into this one)."""
            for _ in attn_unit_gen(h, half, list(fillers), per_qt_sink):
                if guest is not None:
                    drive(guest)

        # ---- output projection ------------------------------------------
        osb_map = {}

        def outproj_chunk(t_, n, tail=False):
            if t_ not in osb_map:
                osb_map[t_] = (out_pool.tile([128, C], F32, tag="osb", name="osb"),
                               set())
            osb, done = osb_map[t_]
            done.add(n)
            op = wk_ps.tile([128, 512], F32, tag="pp", name="op")
            nc.tensor.matmul(op[:],
                             ao[0][:, 128 * t_:128 * (t_ + 1)],
                             wo_t[0][:, 512 * n:512 * (n + 1)],
                             start=True, stop=False)
            nc.tensor.matmul(op[:],
                             ao[1][:, 128 * t_:128 * (t_ + 1)],
                             wo_t[1][:, 512 * n:512 * (n + 1)],
                             start=False, stop=True)
            if tail and n == 1:
                # Act is idle in the drain tail; split engines + chunked DMA
                # to shorten the critical path
                nc.scalar.copy(osb[:, 512 * n:512 * (n + 1)], op[:])
            else:
                nc.vector.tensor_copy(osb[:, 512 * n:512 * (n + 1)], op[:])
            if tail:
                nc.sync.dma_start(out[128 * t_:128 * (t_ + 1), 512 * n:512 * (n + 1)],
                                  osb[:, 512 * n:512 * (n + 1)])
            elif len(done) == 2:
                nc.sync.dma_start(out[128 * t_:128 * (t_ + 1), :], osb[:])
            if len(done) == 2:
                del osb_map[t_]

        def outproj_tile(t_, tail=False):
            outproj_chunk(t_, 0, tail)
            outproj_chunk(t_, 1, tail)

        def pg(m, n, eng="act"):
            return lambda: proj_group(m, n, eng)

        def vt(t_, eng="act"):
            return lambda: vproj_tile(t_, eng)

        def oc(t_, n):
            return lambda: outproj_chunk(t_, n)

        # ---- schedule ----------------------------------------------------
        # prologue: the first four projection groups run k-interleaved so PE
        # consumes each (xt[k], wqkv[k]) DMA pair the moment it lands,
        # accumulating into four concurrent PSUM regions (scores pool is
        # still free). V tiles 0-5 follow while tables stream in.
        pro = [(0, 0), (1, 0), (0, 1), (1, 1)]
        pps = [(sc_ps if i < 3 else wk_ps).tile([128, 512], F32,
                                                tag="sc" if i < 3 else "pp",
                                                name=f"pp{i}")
               for i in range(4)]
        for k in range(CK):
            for (m, n), pp in zip(pro, pps):
                nc.tensor.matmul(pp[:], wqk_t[k][:, 128 * m:128 * (m + 1)],
                                 xt_t[k][:, 512 * n:512 * (n + 1)],
                                 start=(k == 0), stop=(k == CK - 1))

        def drain_pro(i):
            m, n = pro[i]
            nc.scalar.copy(qkt[m][n // 2][:, 512 * (n % 2):512 * (n % 2 + 1)],
                           pps[i][:])
            rope_pending.append((m, n))

        drain_pro(0)
        drain_pro(1)
        vproj_tile(0, flush=False)
        vproj_tile(1, flush=False)
        drain_pro(2)
        drain_pro(3)
        vproj_tile(2)   # flushes the four prologue ropes
        vproj_tile(3)
        vproj_tile(4)
        vproj_tile(5)

        # phase 1: remaining projections woven into the half-0 attention
        # units (PSUM drains on Act, which has slack here). The first 8 kv
        # strips of heads 0/1 of the second query half ride along as guests:
        # their exp runs in phase-1 Act slack while their PE-heavy tails
        # stay in phase 2.
        attn_unit(0, 0, [vt(6), vt(7), pg(2, 0), pg(2, 1)])
        xt_dma(1)
        attn_unit(1, 0, [pg(3, 0), pg(3, 1), pg(0, 2), pg(1, 2), pg(0, 3), pg(1, 3)])
        f01 = [vt(8, "dve"), vt(9, "dve"), vt(12, "dve"), vt(13, "dve"),
               vt(14, "dve"), vt(15, "dve"), pg(2, 2, "dve"), pg(2, 3, "dve")]
        f11 = [vt(10, "dve"), vt(11, "dve"), pg(3, 2, "dve"), pg(3, 3, "dve")]
        g01 = attn_unit_gen(0, 1, f01)
        g11 = attn_unit_gen(1, 1, f11)
        attn_unit(2, 0, [flush_rope], guest=g01)
        attn_unit(3, 0, [], guest=g11)
        flush_rope()

        # phase 2: Act is saturated by exp; the deferred q/k projections and
        # the output projection keep PE fed between strips (drains on DVE).
        f01 += [flush_rope, oc(0, 0), oc(0, 1), oc(1, 0)]
        f11 += [oc(1, 1), oc(2, 0), oc(2, 1), oc(3, 0), oc(3, 1)]
        alive = True
        while alive:
            alive = drive(g01)
            alive = drive(g11) or alive
        # last head pair also interleaves: Act is the wall here, so pack both
        # units' PE work against one continuous exp stream
        f21 = [oc(4, 0), oc(4, 1), oc(5, 0), oc(5, 1),
               oc(6, 0), oc(6, 1), oc(7, 0), oc(7, 1)]
        g21 = attn_unit_gen(2, 1, f21)
        g31 = attn_unit_gen(3, 1, [],
                            per_qt_sink=lambda qt: outproj_tile(qt, tail=(qt >= 12)))
        alive = True
        while alive:
            alive = drive(g21)
            alive = drive(g31) or alive

    nc.finalize()
    return nc


_NC = None


def _get_nc():
    global _NC
    if _NC is None:
        _NC = build_nc()
    return _NC


def _host_tables():
    import ml_dtypes
    bf16 = ml_dtypes.bfloat16
    inv_freq = 1.0 / (10000.0 ** (np.arange(0, Dh, 2, dtype=np.float32) / Dh))  # [32]
    t = np.arange(T, dtype=np.float32)
    freqs = t[:, None] * inv_freq[None, :]                  # [T, 32]
    emb = np.concatenate([freqs, freqs], axis=-1)           # [T, 64]
    cos = np.cos(emb).T.astype(np.float32)                  # [64, T]
    sin = np.sin(emb).T.astype(np.float32)                  # [64, T]
    sin_signed = sin.copy()
    sin_signed[0:32, :] *= -1.0                             # rotate_half sign fold
    cosT = np.concatenate([cos, cos], axis=0).astype(bf16)  # [128, T] two head-halves
    sinT = np.ascontiguousarray(np.concatenate([sin_signed, sin_signed], axis=0))
    maskT = np.triu(np.ones((128, 128), np.float32)).astype(bf16)  # keep where k <= q
    identT = np.eye(128, dtype=np.float32).astype(bf16)
    sigma = np.empty(64, np.int64)
    sigma[0:32] = 2 * np.arange(32) + 1
    sigma[32:64] = 2 * np.arange(32)
    R = np.zeros((128, 128), np.float32)
    for hh in range(2):
        for d in range(64):
            R[64 * hh + d, 64 * hh + sigma[d]] = 1.0
    rotT = np.ascontiguousarray(R.T).astype(bf16)
    return cosT, sinT, maskT, identT, rotT


def kernel(x, w_qkv, w_out):
    import ml_dtypes
    bf16 = ml_dtypes.bfloat16
    x = np.asarray(x, dtype=np.float32)
    w_qkv = np.asarray(w_qkv, dtype=np.float32)
    w_out = np.asarray(w_out, dtype=np.float32)
    nc = _get_nc()
    cosT, sinT, maskT, identT, rotT = _host_tables()

    in_maps = []
    for core in range(N_CORES):
        b = core // 4
        g = core % 4
        heads = [4 * g + l for l in range(HL)]
        qcols = [w_qkv[:, 64 * h:64 * (h + 1)] for h in heads]
        kcols = [w_qkv[:, C + 64 * h:C + 64 * (h + 1)] for h in heads]
        vcols = [w_qkv[:, 2 * C + 64 * h:2 * C + 64 * (h + 1)] for h in heads]
        # m-tiles: Q01 | K01 | Q23 | K23
        wqkv_loc = np.concatenate(
            [qcols[0], qcols[1], kcols[0], kcols[1], qcols[2], qcols[3], kcols[2], kcols[3]]
            + vcols, axis=1).astype(bf16)                    # [C, 768]
        wo_loc = np.concatenate([w_out[64 * h:64 * (h + 1), :] for h in heads],
                                axis=0).astype(bf16)
        in_maps.append({
            "xt": np.ascontiguousarray(x[b].T).astype(bf16),  # [C, T]
            "wqkv": wqkv_loc,
            "wo": wo_loc,
            "cosT": cosT, "sinT": sinT, "maskT": maskT,
            "identT": identT, "rotT": rotT,
        })

    res = run_bass_kernel_spmd(nc, in_maps, core_ids=list(range(N_CORES)))
    out_arr = np.zeros((B, T, C), np.float32)
    for core in range(N_CORES):
        out_arr[core // 4] += res.results[core]["out"]
    return out_arr


# revision 34
# speedup vs baseline: 1.1083x; 1.0119x over previous
"""Multi-head self-attention (RoPE, causal) Trainium2 kernel, 8-way sharded.

Sharding: data-parallel over batch (B=2) x tensor-parallel over head groups
(16 heads -> 4 groups of 4). Core c handles batch c//4, heads 4*(c%4)..+4.
Each core computes q/k/v projections for its heads, RoPE, causal-softmax
attention, and a Megatron-style row-parallel partial of the output
projection; the host sums the 4 partials per batch.

Device dataflow (all matmul operands bf16, accumulation f32 in PSUM):
- scores are computed transposed (scores^T[kpos, q]) per 128-row kv strip,
  exp'd in one Activation op per strip into a bf16 p tile that persists for
  the head-half; causal mask is a bf16 multiply on the diagonal block only.
- attn@V runs with queries on PSUM partitions: per q-tile one contiguous
  burst of [128q x 65] matmuls accumulates p^T V over the kv strips (the
  65th V column is ones so the softmax denominator rides along; PSUM allows
  one pending accumulation group per 2KB bank, hence the burst form). This
  halves PE column count vs. streaming q on the free axis, and
  normalization becomes a native per-partition tensor_scalar multiply.
- per-q-tile PE transposes restore the [channels, q] layout the output
  projection needs as its stationary operand.
- RoPE: rotate_half is a PE permutation matmul; the sign lives in the sin
  table; the elementwise combine is split across DVE/gpsimd.
- work is phase-balanced against the Activation engine (exp is ~60us and
  binds the second query half): V projections for the second half and the
  tail head-pair q/k projections are deferred into the second half as PE
  filler, woven between attention strips.
"""
import sys
for _p in ("/opt/trn_rl_repo",):
    if _p not in sys.path:
        sys.path.insert(0, _p)

import numpy as np
from contextlib import ExitStack

import concourse.bacc as bacc
import concourse.mybir as mybir
import concourse.tile as tile
from concourse.bass_utils import run_bass_kernel_spmd

F32 = mybir.dt.float32
F32R = mybir.dt.float32r
BF16 = mybir.dt.bfloat16
AF = mybir.ActivationFunctionType

B, T, C = 2, 2048, 1024
H, Dh = 16, 64
HL = 4                      # heads per core
CK = C // 128               # 8 contraction k-tiles for projections
TTL = T // 128              # 16 T-tiles / kv k-tiles
HT = T // 2                 # 1024, the attention q-half width
N_CORES = 8


def build_nc():
    nc = bacc.Bacc("TRN2", target_bir_lowering=False, debug=False, num_devices=N_CORES)

    xt = nc.declare_dram_parameter("xt", [C, T], BF16, isOutput=False)
    wqkv = nc.declare_dram_parameter("wqkv", [C, 4 * 128 + HL * Dh], BF16, isOutput=False)
    wo = nc.declare_dram_parameter("wo", [HL * Dh, C], BF16, isOutput=False)
    cosT = nc.declare_dram_parameter("cosT", [128, T], BF16, isOutput=False)
    sinT = nc.declare_dram_parameter("sinT", [128, T], F32R, isOutput=False)
    maskT = nc.declare_dram_parameter("maskT", [128, 128], BF16, isOutput=False)
    identT = nc.declare_dram_parameter("identT", [128, 128], BF16, isOutput=False)
    rotT = nc.declare_dram_parameter("rotT", [128, 128], BF16, isOutput=False)
    out = nc.declare_dram_parameter("out", [T, C], F32, isOutput=True)

    with nc.allow_low_precision("bf16 attention pipeline"), \
         tile.TileContext(nc) as tc, ExitStack() as octx:
        pool = lambda *a, **kw: octx.enter_context(tc.tile_pool(*a, **kw))
        consts = pool(name="consts", bufs=1)
        v_pool = pool(name="v", bufs=1)
        qkt_pool = pool(name="qkt", bufs=1)
        ao_pool = pool(name="ao", bufs=1)
        p_pool = pool(name="pb", bufs=2)
        avn_pool = pool(name="avnp", bufs=3)
        rec_pool = pool(name="recp", bufs=6)
        wo_pool = pool(name="wop", bufs=1)
        xt_pool = pool(name="xtp", bufs=1)
        wqk_pool = pool(name="wqkp", bufs=1)
        rtab_pool = pool(name="ropetab", bufs=1)
        rtmp_pool = pool(name="ropetmp", bufs=2)
        out_pool = pool(name="outsb", bufs=3)
        # PSUM: 3x [128,1024] scores (6 banks) + 2 shared work banks that
        # cycle projection drains, attn@V burst accumulators, transposes and
        # output-projection tiles (every tile's accesses are emitted
        # contiguously, so slot reuse never deadlocks)
        sc_ps = pool(name="scps", bufs=3, space="PSUM")
        wk_ps = pool(name="wkps", bufs=2, space="PSUM")

        mask_t = consts.tile([128, 128], BF16, tag="mask")
        ident_t = consts.tile([128, 128], BF16, tag="ident")
        rotT_t = consts.tile([128, 128], BF16, tag="rotT")

        vext_t = v_pool.tile([128, TTL, HL, Dh + 1], BF16, tag="vext", name="vext")
        vext = [vext_t[:, t_] for t_ in range(TTL)]
        # qkt[mt][half]: mt 0=Q heads01, 1=K heads01, 2=Q heads23, 3=K heads23
        qkt = [[qkt_pool.tile([128, HT], BF16, tag=f"qkt{m}_{hf}", name=f"qkt{m}_{hf}")
                for hf in range(2)] for m in range(4)]
        # ao[pair]: [128 ch (2 heads x 64), T] attention output, transposed
        ao = [ao_pool.tile([128, T], BF16, tag=f"ao{i}", name=f"ao{i}") for i in range(2)]
        wo_t = [wo_pool.tile([128, C], BF16, tag=f"wo{i}", name=f"wo{i}")
                for i in range(2)]
        wqkv_t = [wqk_pool.tile([128, 512 + HL * Dh], BF16, tag=f"wqkv{k}", name=f"wqkv{k}")
                  for k in range(CK)]
        wqk_t = [w[:, 0:512] for w in wqkv_t]
        wv_t = [w[:, 512:512 + HL * Dh] for w in wqkv_t]
        xt_t = [xt_pool.tile([128, T], BF16, tag=f"xt{k}", name=f"xt{k}")
                for k in range(CK)]
        cos_t = rtab_pool.tile([128, T], BF16, tag="cos")
        sin_t = rtab_pool.tile([128, T], F32R, tag="sin")

        state = {"avn": None}

        # ---- input DMA -------------------------------------------------
        # every DMA pays ~625ns on the shared HWDGE descriptor generator and
        # the transfer bus is ~360GB/s shared, so favor few transfers,
        # ordered exactly by first consumption.
        for k in range(CK):
            nc.sync.dma_start(xt_t[k][:, 0:HT], xt[128 * k:128 * (k + 1), 0:HT])
            nc.sync.dma_start(wqkv_t[k][:, 0:512], wqkv[128 * k:128 * (k + 1), 0:512])
        for k in range(CK):   # V weight columns, for the prologue vprojs
            nc.sync.dma_start(wqkv_t[k][:, 512:768], wqkv[128 * k:128 * (k + 1), 512:768])
        nc.sync.dma_start(rotT_t[:], rotT[:])
        nc.sync.dma_start(cos_t[:], cosT[:])
        nc.sync.dma_start(sin_t[:, 0:HT], sinT[:, 0:HT])
        nc.sync.dma_start(mask_t[:], maskT[:])
        for k in range(CK):   # second query half of x, for the half-1 q/k
            nc.sync.dma_start(xt_t[k][:, HT:T], xt[128 * k:128 * (k + 1), HT:T])
        nc.sync.dma_start(sin_t[:, HT:T], sinT[:, HT:T])
        nc.sync.dma_start(ident_t[:], identT[:])
        for i in range(2):
            nc.sync.dma_start(wo_t[i][:], wo[128 * i:128 * (i + 1), :])
        # the softmax-denominator ones column of V, once for all kv tiles
        nc.gpsimd.memset(vext_t[:, :, :, Dh:Dh + 1], 1.0)

        # ---- projections + RoPE ----------------------------------------
        rope_pending = []

        def emit_rope(m, n):
            """rotate-half via a PE permutation matmul, then the cos/sin
            elementwise combine. Emitted one projection group late so the
            PSUM->SBUF drain has completed."""
            dst = qkt[m][n // 2]
            src = dst[:, 512 * (n % 2):512 * (n % 2 + 1)]
            rps = sc_ps.tile([128, 512], F32, tag="sc", name="rps")
            nc.tensor.matmul(rps[:], rotT_t[:], src, start=True, stop=True)
            rot = rtmp_pool.tile([128, 512], BF16, tag="rot", name="rot")
            nc.vector.tensor_mul(rot[:], rps[:].bitcast(F32R),
                                 sin_t[:, 512 * n:512 * (n + 1)])
            nc.gpsimd.tensor_mul(src, src, cos_t[:, 512 * n:512 * (n + 1)])
            nc.vector.tensor_add(src, src, rot[:])

        def flush_rope():
            while rope_pending:
                emit_rope(*rope_pending.pop(0))

        def proj_group(m, n, eng="act"):
            pp = wk_ps.tile([128, 512], F32, tag="pp", name="pp")
            for k in range(CK):
                nc.tensor.matmul(pp[:], wqk_t[k][:, 128 * m:128 * (m + 1)],
                                 xt_t[k][:, 512 * n:512 * (n + 1)],
                                 start=(k == 0), stop=(k == CK - 1))
            dst = qkt[m][n // 2]
            dsl = dst[:, 512 * (n % 2):512 * (n % 2 + 1)]
            if eng == "act":
                nc.scalar.copy(dsl, pp[:])
            else:
                nc.vector.tensor_copy(dsl, pp[:])
            pending = rope_pending[:]
            rope_pending.clear()
            rope_pending.append((m, n))
            for pmn in pending:
                emit_rope(*pmn)

        def vproj_tile(t_, eng="act", flush=True):
            if flush:
                flush_rope()
            vp = wk_ps.tile([128, HL * Dh], F32, tag="pp", name="vp")
            for k in range(CK):
                nc.tensor.matmul(vp[:], xt_t[k][:, 128 * t_:128 * (t_ + 1)], wv_t[k][:],
                                 start=(k == 0), stop=(k == CK - 1))
            src = vp[:].rearrange("p (h d) -> p h d", h=HL)
            if eng == "act":
                nc.scalar.copy(vext[t_][:, :, 0:Dh], src)
            else:
                nc.vector.tensor_copy(vext[t_][:, :, 0:Dh], src)

        # ---- attention ---------------------------------------------------
        def attn_unit_gen(h, half, fillers, per_qt_sink=None):
            """scores^T/exp/mask + [q,ch]-oriented attn@V for head h, query
            half `half`, as a generator yielding once per kv strip (so units
            can be interleaved). `fillers` is a MUTABLE list; one closure is
            popped per strip to keep PE fed while the softmax pipeline runs,
            and callers may append more mid-flight. `per_qt_sink(qt)` (if
            set) is called right after q-tile qt is drained+transposed."""
            hp, hl = h // 2, h % 2
            qrmt, krmt = (0, 1) if h < 2 else (2, 3)
            pr = 64 * hl
            q_lo = HT * half
            qt0 = 8 * half
            n_strips = 8 if half == 0 else 16
            per_qt = per_qt_sink is not None
            strips = {}

            if hl == 0:
                avn = avn_pool.tile([128, 8, 128], BF16, tag="avn", name="avn")
                state[f"avn{hp}_{half}"] = avn
            else:
                avn = state[f"avn{hp}_{half}"]

            def transpose_qt(lqt, act=False):
                """[128 q, 128 ch] -> ao[hp][:, qcols] via PE transpose."""
                tt = wk_ps.tile([128, 128], BF16, tag="pp", name="tt")
                nc.tensor.transpose(tt[:], avn[:, lqt, :], ident_t[:])
                qtg = qt0 + lqt
                dst = ao[hp][:, 128 * qtg:128 * (qtg + 1)]
                if act:
                    nc.scalar.copy(dst, tt[:])
                else:
                    nc.vector.tensor_copy(dst, tt[:])

            tail_pending = []

            def flush_tail(lqt):
                """transpose + sink one strip behind the drain chain, so the
                cross-engine recip/normalize latency never blocks PE."""
                in_tail = lqt + qt0 >= 13
                transpose_qt(lqt, act=in_tail)
                per_qt_sink(qt0 + lqt)

            def emit_burst(qt):
                lqt = qt - qt0
                av = sc_ps.tile([128, Dh + 1], F32, tag="sc", name="av")
                for m2 in range(qt + 1):
                    p_, cs_ = strips[m2]
                    lq = 128 * qt - cs_
                    nc.tensor.matmul(av[:], p_[:, lq:lq + 128], vext[m2][:, h, :],
                                     start=(m2 == 0), stop=(m2 == qt))
                rec = rec_pool.tile([128, 1], F32, tag="rec", name="rec")
                nc.vector.reciprocal(rec[:], av[:, Dh:Dh + 1])
                # normalize out of PSUM into avn; Act takes over in the drain
                # tail where exp is finished (gpsimd cannot touch PSUM)
                if per_qt and qt >= 13:
                    nc.scalar.activation(avn[:, lqt, pr:pr + 64], av[:, 0:Dh],
                                         AF.Copy, scale=rec[:])
                else:
                    nc.vector.tensor_scalar_mul(
                        avn[:, lqt, pr:pr + 64], av[:, 0:Dh], rec[:])
                if per_qt:
                    tail_pending.append(lqt)
                    if len(tail_pending) >= 2:
                        flush_tail(tail_pending.pop(0))

            pending = None
            for m in range(n_strips):
                cs = max(q_lo, 128 * m)
                W = q_lo + HT - cs
                kr_t = qkt[krmt][m // 8]
                kc = 128 * m - HT * (m // 8)
                sc = sc_ps.tile([128, W], F32, tag="sc", name="sc")
                j = 0
                while 512 * j < W:
                    n = min(512, W - 512 * j)
                    qc = (cs - q_lo) + 512 * j
                    nc.tensor.matmul(
                        sc[:, 512 * j:512 * j + n],
                        kr_t[pr:pr + 64, kc:kc + 128],
                        qkt[qrmt][half][pr:pr + 64, qc:qc + n],
                        start=True, stop=True)
                    j += 1
                # strips of the second half overlap three units in flight
                p = p_pool.tile([128, W], BF16, tag=f"p{m}", name=f"p{m}",
                                bufs=3)
                nc.scalar.activation(p[:], sc[:, 0:W], AF.Exp, scale=0.125)
                if cs == 128 * m:
                    # gpsimd: all-SBUF bf16, keeps DVE free for PSUM drains
                    nc.gpsimd.tensor_mul(p[:, 0:128], p[:, 0:128], mask_t[:])
                strips[m] = (p, cs)
                if pending is not None:
                    emit_burst(pending)
                    pending = None
                if m >= qt0:
                    pending = m
                if m >= 1 and fillers:
                    fillers.pop(0)()
                yield
            if pending is not None:
                emit_burst(pending)
            while tail_pending:
                flush_tail(tail_pending.pop(0))
            if hl == 1 and not per_qt:
                for lqt in range(8):
                    transpose_qt(lqt)
            while fillers:
                fillers.pop(0)()

        def drive(gen):
            try:
                next(gen)
                return True
            except StopIteration:
                return False

        def attn_unit(h, half, fillers=(), per_qt_sink=None, guest=None):
            """run a unit to completion, advancing `guest` one strip per own
            strip (interleaves a later unit's Act work into this one)."""
            for _ in attn_unit_gen(h, half, list(fillers), per_qt_sink):
                if guest is not None:
                    drive(guest)

        # ---- output projection ------------------------------------------
        osb_map = {}

        def outproj_chunk(t_, n, tail=False):
            if t_ not in osb_map:
                osb_map[t_] = (out_pool.tile([128, C], F32, tag="osb", name="osb"),
                               set())
            osb, done = osb_map[t_]
            done.add(n)
            op = wk_ps.tile([128, 512], F32, tag="pp", name="op")
            nc.tensor.matmul(op[:],
                             ao[0][:, 128 * t_:128 * (t_ + 1)],
                             wo_t[0][:, 512 * n:512 * (n + 1)],
                             start=True, stop=False)
            nc.tensor.matmul(op[:],
                             ao[1][:, 128 * t_:128 * (t_ + 1)],
                             wo_t[1][:, 512 * n:512 * (n + 1)],
                             start=False, stop=True)
            if tail and n == 1:
                # Act is idle in the drain tail; split engines + chunked DMA
                # to shorten the critical path
                nc.scalar.copy(osb[:, 512 * n:512 * (n + 1)], op[:])
            else:
                nc.vector.tensor_copy(osb[:, 512 * n:512 * (n + 1)], op[:])
            if tail:
                nc.sync.dma_start(out[128 * t_:128 * (t_ + 1), 512 * n:512 * (n + 1)],
                                  osb[:, 512 * n:512 * (n + 1)])
            elif len(done) == 2:
                nc.sync.dma_start(out[128 * t_:128 * (t_ + 1), :], osb[:])
            if len(done) == 2:
                del osb_map[t_]

        def outproj_tile(t_, tail=False):
            outproj_chunk(t_, 0, tail)
            outproj_chunk(t_, 1, tail)

        def pg(m, n, eng="act"):
            return lambda: proj_group(m, n, eng)

        def vt(t_, eng="act"):
            return lambda: vproj_tile(t_, eng)

        def oc(t_, n):
            return lambda: outproj_chunk(t_, n)

        # ---- schedule ----------------------------------------------------
        # prologue: the first four projection groups run k-interleaved so PE
        # consumes each (xt[k], wqkv[k]) DMA pair the moment it lands,
        # accumulating into four concurrent PSUM regions (scores pool is
        # still free). V tiles 0-5 follow while tables stream in.
        pro = [(0, 0), (1, 0), (0, 1), (1, 1)]
        pps = [(sc_ps if i < 3 else wk_ps).tile([128, 512], F32,
                                                tag="sc" if i < 3 else "pp",
                                                name=f"pp{i}")
               for i in range(4)]
        for k in range(CK):
            for (m, n), pp in zip(pro, pps):
                nc.tensor.matmul(pp[:], wqk_t[k][:, 128 * m:128 * (m + 1)],
                                 xt_t[k][:, 512 * n:512 * (n + 1)],
                                 start=(k == 0), stop=(k == CK - 1))

        def drain_pro(i):
            m, n = pro[i]
            nc.scalar.copy(qkt[m][n // 2][:, 512 * (n % 2):512 * (n % 2 + 1)],
                           pps[i][:])
            rope_pending.append((m, n))

        drain_pro(0)
        drain_pro(1)
        vproj_tile(0, flush=False)
        vproj_tile(1, flush=False)
        drain_pro(2)
        drain_pro(3)
        vproj_tile(2)   # flushes the four prologue ropes
        vproj_tile(3)
        vproj_tile(4)
        vproj_tile(5)

        # phase 1: remaining projections woven into the half-0 attention
        # units (PSUM drains on Act, which has slack here). Second-half
        # units ride along as guests as soon as their q/k tiles are roped:
        # their exp fills phase-1 Act slack, their PE-heavy burst tails
        # interleave later.
        attn_unit(0, 0, [vt(6), vt(7), pg(2, 0), pg(2, 1)])
        attn_unit(1, 0, [pg(3, 0), pg(3, 1), pg(0, 2), pg(1, 2), pg(0, 3), pg(1, 3)])
        f01 = [vt(8, "dve"), vt(9, "dve"), vt(10, "dve"), vt(11, "dve"),
               vt(12, "dve"), vt(13, "dve"), vt(14, "dve"), vt(15, "dve")]
        f11 = [pg(2, 2, "dve"), pg(2, 3, "dve"), flush_rope,
               pg(3, 2, "dve"), pg(3, 3, "dve")]
        g01 = attn_unit_gen(0, 1, f01)
        g11 = attn_unit_gen(1, 1, f11)
        attn_unit(2, 0, [flush_rope], guest=g01)
        attn_unit(3, 0, [], guest=g11)
        flush_rope()

        # phase 2: staggered 3-wide round-robin keeps one continuous exp
        # stream on Act while the deferred projections and the output
        # projection keep PE fed (drains on DVE).
        f11 += [flush_rope, oc(0, 0), oc(0, 1)]
        f21 = [oc(1, 0), oc(1, 1), oc(2, 0), oc(2, 1),
               oc(3, 0), oc(3, 1), oc(4, 0), oc(4, 1)]
        f31 = [oc(5, 0), oc(5, 1), oc(6, 0), oc(6, 1), oc(7, 0), oc(7, 1)]
        g21 = attn_unit_gen(2, 1, f21)
        g31 = attn_unit_gen(3, 1, f31,
                            per_qt_sink=lambda qt: outproj_tile(qt, tail=(qt >= 12)))
        active = [g01, g11, g21]
        queue = [g31]
        while active:
            for g in list(active):
                if not drive(g):
                    active.remove(g)
                    if queue:
                        active.append(queue.pop(0))

    nc.finalize()
    return nc


_NC = None


def _get_nc():
    global _NC
    if _NC is None:
        _NC = build_nc()
    return _NC


def _host_tables():
    import ml_dtypes
    bf16 = ml_dtypes.bfloat16
    inv_freq = 1.0 / (10000.0 ** (np.arange(0, Dh, 2, dtype=np.float32) / Dh))  # [32]
    t = np.arange(T, dtype=np.float32)
    freqs = t[:, None] * inv_freq[None, :]                  # [T, 32]
    emb = np.concatenate([freqs, freqs], axis=-1)           # [T, 64]
    cos = np.cos(emb).T.astype(np.float32)                  # [64, T]
    sin = np.sin(emb).T.astype(np.float32)                  # [64, T]
    sin_signed = sin.copy()
    sin_signed[0:32, :] *= -1.0                             # rotate_half sign fold
    cosT = np.concatenate([cos, cos], axis=0).astype(bf16)  # [128, T] two head-halves
    sinT = np.ascontiguousarray(np.concatenate([sin_signed, sin_signed], axis=0))
    maskT = np.triu(np.ones((128, 128), np.float32)).astype(bf16)  # keep where k <= q
    identT = np.eye(128, dtype=np.float32).astype(bf16)
    sigma = np.empty(64, np.int64)
    sigma[0:32] = 2 * np.arange(32) + 1
    sigma[32:64] = 2 * np.arange(32)
    R = np.zeros((128, 128), np.float32)
    for hh in range(2):
        for d in range(64):
            R[64 * hh + d, 64 * hh + sigma[d]] = 1.0
    rotT = np.ascontiguousarray(R.T).astype(bf16)
    return cosT, sinT, maskT, identT, rotT


def kernel(x, w_qkv, w_out):
    import ml_dtypes
    bf16 = ml_dtypes.bfloat16
    x = np.asarray(x, dtype=np.float32)
    w_qkv = np.asarray(w_qkv, dtype=np.float32)
    w_out = np.asarray(w_out, dtype=np.float32)
    nc = _get_nc()
    cosT, sinT, maskT, identT, rotT = _host_tables()

    in_maps = []
    for core in range(N_CORES):
        b = core // 4
        g = core % 4
        heads = [4 * g + l for l in range(HL)]
        qcols = [w_qkv[:, 64 * h:64 * (h + 1)] for h in heads]
        kcols = [w_qkv[:, C + 64 * h:C + 64 * (h + 1)] for h in heads]
        vcols = [w_qkv[:, 2 * C + 64 * h:2 * C + 64 * (h + 1)] for h in heads]
        # m-tiles: Q01 | K01 | Q23 | K23
        wqkv_loc = np.concatenate(
            [qcols[0], qcols[1], kcols[0], kcols[1], qcols[2], qcols[3], kcols[2], kcols[3]]
            + vcols, axis=1).astype(bf16)                    # [C, 768]
        wo_loc = np.concatenate([w_out[64 * h:64 * (h + 1), :] for h in heads],
                                axis=0).astype(bf16)
        in_maps.append({
            "xt": np.ascontiguousarray(x[b].T).astype(bf16),  # [C, T]
            "wqkv": wqkv_loc,
            "wo": wo_loc,
            "cosT": cosT, "sinT": sinT, "maskT": maskT,
            "identT": identT, "rotT": rotT,
        })

    res = run_bass_kernel_spmd(nc, in_maps, core_ids=list(range(N_CORES)))
    out_arr = np.zeros((B, T, C), np.float32)
    for core in range(N_CORES):
        out_arr[core // 4] += res.results[core]["out"]
    return out_arr


# revision 36
# speedup vs baseline: 1.1127x; 1.0040x over previous
"""Multi-head self-attention (RoPE, causal) Trainium2 kernel, 8-way sharded.

Sharding: data-parallel over batch (B=2) x tensor-parallel over head groups
(16 heads -> 4 groups of 4). Core c handles batch c//4, heads 4*(c%4)..+4.
Each core computes q/k/v projections for its heads, RoPE, causal-softmax
attention, and a Megatron-style row-parallel partial of the output
projection; the host sums the 4 partials per batch.

Device dataflow (all matmul operands bf16, accumulation f32 in PSUM):
- scores are computed transposed (scores^T[kpos, q]) per 128-row kv strip,
  exp'd in one Activation op per strip into a bf16 p tile that persists for
  the head-half; causal mask is a bf16 multiply on the diagonal block only.
- attn@V runs with queries on PSUM partitions: per q-tile one contiguous
  burst of [128q x 65] matmuls accumulates p^T V over the kv strips (the
  65th V column is ones so the softmax denominator rides along; PSUM allows
  one pending accumulation group per 2KB bank, hence the burst form). This
  halves PE column count vs. streaming q on the free axis, and
  normalization becomes a native per-partition tensor_scalar multiply.
- per-q-tile PE transposes restore the [channels, q] layout the output
  projection needs as its stationary operand.
- RoPE: rotate_half is a PE permutation matmul; the sign lives in the sin
  table; the elementwise combine is split across DVE/gpsimd.
- work is phase-balanced against the Activation engine (exp is ~60us and
  binds the second query half): V projections for the second half and the
  tail head-pair q/k projections are deferred into the second half as PE
  filler, woven between attention strips.
"""
import sys
for _p in ("/opt/trn_rl_repo",):
    if _p not in sys.path:
        sys.path.insert(0, _p)

import numpy as np
from contextlib import ExitStack

import concourse.bacc as bacc
import concourse.mybir as mybir
import concourse.tile as tile
from concourse.bass_utils import run_bass_kernel_spmd

F32 = mybir.dt.float32
F32R = mybir.dt.float32r
BF16 = mybir.dt.bfloat16
AF = mybir.ActivationFunctionType

B, T, C = 2, 2048, 1024
H, Dh = 16, 64
HL = 4                      # heads per core
CK = C // 128               # 8 contraction k-tiles for projections
TTL = T // 128              # 16 T-tiles / kv k-tiles
HT = T // 2                 # 1024, the attention q-half width
N_CORES = 8


def build_nc():
    nc = bacc.Bacc("TRN2", target_bir_lowering=False, debug=False, num_devices=N_CORES)

    xt = nc.declare_dram_parameter("xt", [C, T], BF16, isOutput=False)
    wqkv = nc.declare_dram_parameter("wqkv", [C, 4 * 128 + HL * Dh], BF16, isOutput=False)
    wo = nc.declare_dram_parameter("wo", [HL * Dh, C], BF16, isOutput=False)
    cosT = nc.declare_dram_parameter("cosT", [128, T], BF16, isOutput=False)
    sinT = nc.declare_dram_parameter("sinT", [128, T], F32R, isOutput=False)
    maskT = nc.declare_dram_parameter("maskT", [128, 128], BF16, isOutput=False)
    identT = nc.declare_dram_parameter("identT", [128, 128], BF16, isOutput=False)
    rotT = nc.declare_dram_parameter("rotT", [128, 128], BF16, isOutput=False)
    out = nc.declare_dram_parameter("out", [T, C], F32, isOutput=True)

    with nc.allow_low_precision("bf16 attention pipeline"), \
         tile.TileContext(nc) as tc, ExitStack() as octx:
        pool = lambda *a, **kw: octx.enter_context(tc.tile_pool(*a, **kw))
        consts = pool(name="consts", bufs=1)
        v_pool = pool(name="v", bufs=1)
        qkt_pool = pool(name="qkt", bufs=1)
        ao_pool = pool(name="ao", bufs=1)
        p_pool = pool(name="pb", bufs=2)
        avn_pool = pool(name="avnp", bufs=3)
        rec_pool = pool(name="recp", bufs=6)
        wo_pool = pool(name="wop", bufs=1)
        xt_pool = pool(name="xtp", bufs=1)
        wqk_pool = pool(name="wqkp", bufs=1)
        rtab_pool = pool(name="ropetab", bufs=1)
        rtmp_pool = pool(name="ropetmp", bufs=2)
        out_pool = pool(name="outsb", bufs=3)
        # PSUM: 3x [128,1024] scores (6 banks) + 2 shared work banks that
        # cycle projection drains, attn@V burst accumulators, transposes and
        # output-projection tiles (every tile's accesses are emitted
        # contiguously, so slot reuse never deadlocks)
        sc_ps = pool(name="scps", bufs=3, space="PSUM")
        wk_ps = pool(name="wkps", bufs=2, space="PSUM")

        mask_t = consts.tile([128, 128], BF16, tag="mask")
        ident_t = consts.tile([128, 128], BF16, tag="ident")
        rotT_t = consts.tile([128, 128], BF16, tag="rotT")

        vext_t = v_pool.tile([128, TTL, HL, Dh + 1], BF16, tag="vext", name="vext")
        vext = [vext_t[:, t_] for t_ in range(TTL)]
        # qkt[mt][half]: mt 0=Q heads01, 1=K heads01, 2=Q heads23, 3=K heads23
        qkt = [[qkt_pool.tile([128, HT], BF16, tag=f"qkt{m}_{hf}", name=f"qkt{m}_{hf}")
                for hf in range(2)] for m in range(4)]
        # ao[pair]: [128 ch (2 heads x 64), T] attention output, transposed
        ao = [ao_pool.tile([128, T], BF16, tag=f"ao{i}", name=f"ao{i}") for i in range(2)]
        wo_t = [wo_pool.tile([128, C], BF16, tag=f"wo{i}", name=f"wo{i}")
                for i in range(2)]
        wqkv_t = [wqk_pool.tile([128, 512 + HL * Dh], BF16, tag=f"wqkv{k}", name=f"wqkv{k}")
                  for k in range(CK)]
        wqk_t = [w[:, 0:512] for w in wqkv_t]
        wv_t = [w[:, 512:512 + HL * Dh] for w in wqkv_t]
        xt_t = [xt_pool.tile([128, T], BF16, tag=f"xt{k}", name=f"xt{k}")
                for k in range(CK)]
        cos_t = rtab_pool.tile([128, T], BF16, tag="cos")
        sin_t = rtab_pool.tile([128, T], F32R, tag="sin")

        state = {"avn": None}

        # ---- input DMA -------------------------------------------------
        # every DMA pays ~625ns on the shared HWDGE descriptor generator and
        # the transfer bus is ~360GB/s shared, so favor few transfers,
        # ordered exactly by first consumption.
        for k in range(CK):
            nc.sync.dma_start(xt_t[k][:, 0:HT], xt[128 * k:128 * (k + 1), 0:HT])
            nc.sync.dma_start(wqkv_t[k][:, 0:512], wqkv[128 * k:128 * (k + 1), 0:512])
        for k in range(CK):   # V weight columns, for the prologue vprojs
            nc.sync.dma_start(wqkv_t[k][:, 512:768], wqkv[128 * k:128 * (k + 1), 512:768])
        nc.sync.dma_start(rotT_t[:], rotT[:])
        nc.sync.dma_start(cos_t[:], cosT[:])
        nc.sync.dma_start(sin_t[:, 0:HT], sinT[:, 0:HT])
        nc.sync.dma_start(mask_t[:], maskT[:])
        for k in range(CK):   # second query half of x, for the half-1 q/k
            nc.sync.dma_start(xt_t[k][:, HT:T], xt[128 * k:128 * (k + 1), HT:T])
        nc.sync.dma_start(sin_t[:, HT:T], sinT[:, HT:T])
        nc.sync.dma_start(ident_t[:], identT[:])
        for i in range(2):
            nc.sync.dma_start(wo_t[i][:], wo[128 * i:128 * (i + 1), :])
        # the softmax-denominator ones column of V, once for all kv tiles
        nc.gpsimd.memset(vext_t[:, :, :, Dh:Dh + 1], 1.0)

        # ---- projections + RoPE ----------------------------------------
        rope_pending = []

        def emit_rope(m, n):
            """rotate-half via a PE permutation matmul, then the cos/sin
            elementwise combine. Emitted one projection group late so the
            PSUM->SBUF drain has completed."""
            dst = qkt[m][n // 2]
            src = dst[:, 512 * (n % 2):512 * (n % 2 + 1)]
            rps = sc_ps.tile([128, 512], F32, tag="sc", name="rps")
            nc.tensor.matmul(rps[:], rotT_t[:], src, start=True, stop=True)
            rot = rtmp_pool.tile([128, 512], BF16, tag="rot", name="rot")
            nc.vector.tensor_mul(rot[:], rps[:].bitcast(F32R),
                                 sin_t[:, 512 * n:512 * (n + 1)])
            nc.gpsimd.tensor_mul(src, src, cos_t[:, 512 * n:512 * (n + 1)])
            nc.vector.tensor_add(src, src, rot[:])

        def flush_rope():
            while rope_pending:
                emit_rope(*rope_pending.pop(0))

        def proj_group(m, n, eng="act"):
            pp = wk_ps.tile([128, 512], F32, tag="pp", name="pp")
            for k in range(CK):
                nc.tensor.matmul(pp[:], wqk_t[k][:, 128 * m:128 * (m + 1)],
                                 xt_t[k][:, 512 * n:512 * (n + 1)],
                                 start=(k == 0), stop=(k == CK - 1))
            dst = qkt[m][n // 2]
            dsl = dst[:, 512 * (n % 2):512 * (n % 2 + 1)]
            if eng == "act":
                nc.scalar.copy(dsl, pp[:])
            else:
                nc.vector.tensor_copy(dsl, pp[:])
            pending = rope_pending[:]
            rope_pending.clear()
            rope_pending.append((m, n))
            for pmn in pending:
                emit_rope(*pmn)

        def vproj_tile(t_, eng="act", flush=True):
            if flush:
                flush_rope()
            vp = wk_ps.tile([128, HL * Dh], F32, tag="pp", name="vp")
            for k in range(CK):
                nc.tensor.matmul(vp[:], xt_t[k][:, 128 * t_:128 * (t_ + 1)], wv_t[k][:],
                                 start=(k == 0), stop=(k == CK - 1))
            src = vp[:].rearrange("p (h d) -> p h d", h=HL)
            if eng == "act":
                nc.scalar.copy(vext[t_][:, :, 0:Dh], src)
            else:
                nc.vector.tensor_copy(vext[t_][:, :, 0:Dh], src)

        # ---- attention ---------------------------------------------------
        def attn_unit_gen(h, half, fillers, per_qt_sink=None):
            """scores^T/exp/mask + [q,ch]-oriented attn@V for head h, query
            half `half`, as a generator yielding once per kv strip (so units
            can be interleaved). `fillers` is a MUTABLE list; one closure is
            popped per strip to keep PE fed while the softmax pipeline runs,
            and callers may append more mid-flight. `per_qt_sink(qt)` (if
            set) is called right after q-tile qt is drained+transposed."""
            hp, hl = h // 2, h % 2
            qrmt, krmt = (0, 1) if h < 2 else (2, 3)
            pr = 64 * hl
            q_lo = HT * half
            qt0 = 8 * half
            n_strips = 8 if half == 0 else 16
            per_qt = per_qt_sink is not None
            strips = {}

            if hl == 0:
                avn = avn_pool.tile([128, 8, 128], BF16, tag="avn", name="avn")
                state[f"avn{hp}_{half}"] = avn
            else:
                avn = state[f"avn{hp}_{half}"]

            def transpose_qt(lqt, act=False):
                """[128 q, 128 ch] -> ao[hp][:, qcols] via PE transpose."""
                tt = wk_ps.tile([128, 128], BF16, tag="pp", name="tt")
                nc.tensor.transpose(tt[:], avn[:, lqt, :], ident_t[:])
                qtg = qt0 + lqt
                dst = ao[hp][:, 128 * qtg:128 * (qtg + 1)]
                if act:
                    nc.scalar.copy(dst, tt[:])
                else:
                    nc.vector.tensor_copy(dst, tt[:])

            tail_pending = []

            def flush_tail(lqt):
                """transpose + sink one strip behind the drain chain, so the
                cross-engine recip/normalize latency never blocks PE."""
                in_tail = lqt + qt0 >= 13
                transpose_qt(lqt, act=in_tail)
                per_qt_sink(qt0 + lqt)

            def emit_burst(qt):
                lqt = qt - qt0
                av = sc_ps.tile([128, Dh + 1], F32, tag="sc", name="av")
                for m2 in range(qt + 1):
                    p_, cs_ = strips[m2]
                    lq = 128 * qt - cs_
                    nc.tensor.matmul(av[:], p_[:, lq:lq + 128], vext[m2][:, h, :],
                                     start=(m2 == 0), stop=(m2 == qt))
                rec = rec_pool.tile([128, 1], F32, tag="rec", name="rec")
                nc.vector.reciprocal(rec[:], av[:, Dh:Dh + 1])
                # normalize out of PSUM into avn (gpsimd cannot touch PSUM)
                nc.vector.tensor_scalar_mul(
                    avn[:, lqt, pr:pr + 64], av[:, 0:Dh], rec[:])
                if per_qt:
                    tail_pending.append(lqt)
                    if len(tail_pending) >= 2:
                        flush_tail(tail_pending.pop(0))

            pending = None
            for m in range(n_strips):
                cs = max(q_lo, 128 * m)
                W = q_lo + HT - cs
                kr_t = qkt[krmt][m // 8]
                kc = 128 * m - HT * (m // 8)
                sc = sc_ps.tile([128, W], F32, tag="sc", name="sc")
                j = 0
                while 512 * j < W:
                    n = min(512, W - 512 * j)
                    qc = (cs - q_lo) + 512 * j
                    nc.tensor.matmul(
                        sc[:, 512 * j:512 * j + n],
                        kr_t[pr:pr + 64, kc:kc + 128],
                        qkt[qrmt][half][pr:pr + 64, qc:qc + n],
                        start=True, stop=True)
                    j += 1
                # strips of the second half overlap three units in flight
                p = p_pool.tile([128, W], BF16, tag=f"p{m}", name=f"p{m}",
                                bufs=3)
                nc.scalar.activation(p[:], sc[:, 0:W], AF.Exp, scale=0.125)
                if cs == 128 * m:
                    # gpsimd: all-SBUF bf16, keeps DVE free for PSUM drains
                    nc.gpsimd.tensor_mul(p[:, 0:128], p[:, 0:128], mask_t[:])
                strips[m] = (p, cs)
                if pending is not None:
                    emit_burst(pending)
                    pending = None
                if m >= qt0:
                    pending = m
                if m >= 1 and fillers:
                    fillers.pop(0)()
                yield
            if pending is not None:
                emit_burst(pending)
            while tail_pending:
                flush_tail(tail_pending.pop(0))
            if hl == 1 and not per_qt:
                for lqt in range(8):
                    transpose_qt(lqt)
            while fillers:
                fillers.pop(0)()

        def drive(gen):
            try:
                next(gen)
                return True
            except StopIteration:
                return False

        def attn_unit(h, half, fillers=(), per_qt_sink=None, guest=None):
            """run a unit to completion, advancing `guest` one strip per own
            strip (interleaves a later unit's Act work into this one)."""
            for _ in attn_unit_gen(h, half, list(fillers), per_qt_sink):
                if guest is not None:
                    drive(guest)

        # ---- output projection ------------------------------------------
        osb_map = {}

        def outproj_chunk(t_, n, tail=False):
            if t_ not in osb_map:
                osb_map[t_] = (out_pool.tile([128, C], F32, tag="osb", name="osb"),
                               set())
            osb, done = osb_map[t_]
            done.add(n)
            op = wk_ps.tile([128, 512], F32, tag="pp", name="op")
            nc.tensor.matmul(op[:],
                             ao[0][:, 128 * t_:128 * (t_ + 1)],
                             wo_t[0][:, 512 * n:512 * (n + 1)],
                             start=True, stop=False)
            nc.tensor.matmul(op[:],
                             ao[1][:, 128 * t_:128 * (t_ + 1)],
                             wo_t[1][:, 512 * n:512 * (n + 1)],
                             start=False, stop=True)
            if tail and n == 1:
                # Act is idle in the drain tail; split engines + chunked DMA
                # to shorten the critical path
                nc.scalar.copy(osb[:, 512 * n:512 * (n + 1)], op[:])
            else:
                nc.vector.tensor_copy(osb[:, 512 * n:512 * (n + 1)], op[:])
            if tail:
                nc.sync.dma_start(out[128 * t_:128 * (t_ + 1), 512 * n:512 * (n + 1)],
                                  osb[:, 512 * n:512 * (n + 1)])
            elif len(done) == 2:
                nc.sync.dma_start(out[128 * t_:128 * (t_ + 1), :], osb[:])
            if len(done) == 2:
                del osb_map[t_]

        def outproj_tile(t_, tail=False):
            outproj_chunk(t_, 0, tail)
            outproj_chunk(t_, 1, tail)

        def pg(m, n, eng="act"):
            return lambda: proj_group(m, n, eng)

        def vt(t_, eng="act"):
            return lambda: vproj_tile(t_, eng)

        def oc(t_, n):
            return lambda: outproj_chunk(t_, n)

        # ---- schedule ----------------------------------------------------
        # prologue: the first four projection groups run k-interleaved so PE
        # consumes each (xt[k], wqkv[k]) DMA pair the moment it lands,
        # accumulating into four concurrent PSUM regions (scores pool is
        # still free). V tiles 0-5 follow while tables stream in.
        pro = [(0, 0), (1, 0), (0, 1), (1, 1)]
        pps = [(sc_ps if i < 3 else wk_ps).tile([128, 512], F32,
                                                tag="sc" if i < 3 else "pp",
                                                name=f"pp{i}")
               for i in range(4)]
        for k in range(CK):
            for (m, n), pp in zip(pro, pps):
                nc.tensor.matmul(pp[:], wqk_t[k][:, 128 * m:128 * (m + 1)],
                                 xt_t[k][:, 512 * n:512 * (n + 1)],
                                 start=(k == 0), stop=(k == CK - 1))

        def drain_pro(i):
            m, n = pro[i]
            nc.scalar.copy(qkt[m][n // 2][:, 512 * (n % 2):512 * (n % 2 + 1)],
                           pps[i][:])
            rope_pending.append((m, n))

        drain_pro(0)
        drain_pro(1)
        vproj_tile(0, flush=False)
        vproj_tile(1, flush=False)
        drain_pro(2)
        drain_pro(3)
        vproj_tile(2)   # flushes the four prologue ropes
        vproj_tile(3)
        vproj_tile(4)
        vproj_tile(5)

        # phase 1: remaining projections woven into the half-0 attention
        # units (PSUM drains on Act, which has slack here). Second-half
        # units ride along as guests as soon as their q/k tiles are roped:
        # their exp fills phase-1 Act slack, their PE-heavy burst tails
        # interleave later.
        attn_unit(0, 0, [vt(6), vt(7), pg(2, 0), pg(2, 1)])
        attn_unit(1, 0, [pg(3, 0), pg(3, 1), pg(0, 2), pg(1, 2), pg(0, 3), pg(1, 3)])
        f01 = [vt(8, "dve"), vt(9, "dve"), vt(10, "dve"), vt(11, "dve"),
               vt(12, "dve"), vt(13, "dve"), vt(14, "dve"), vt(15, "dve")]
        f11 = [pg(2, 2, "dve"), pg(2, 3, "dve"), flush_rope,
               pg(3, 2, "dve"), pg(3, 3, "dve")]
        g01 = attn_unit_gen(0, 1, f01)
        g11 = attn_unit_gen(1, 1, f11)
        attn_unit(2, 0, [flush_rope], guest=g01)
        attn_unit(3, 0, [], guest=g11)
        flush_rope()

        # phase 2: staggered 3-wide round-robin keeps one continuous exp
        # stream on Act while the deferred projections and the output
        # projection keep PE fed (drains on DVE).
        f11 += [flush_rope, oc(0, 0), oc(0, 1)]
        f21 = [oc(1, 0), oc(1, 1), oc(2, 0), oc(2, 1),
               oc(3, 0), oc(3, 1), oc(4, 0), oc(4, 1)]
        f31 = [oc(5, 0), oc(5, 1), oc(6, 0), oc(6, 1), oc(7, 0), oc(7, 1)]
        g21 = attn_unit_gen(2, 1, f21)
        g31 = attn_unit_gen(3, 1, f31,
                            per_qt_sink=lambda qt: outproj_tile(qt, tail=(qt >= 10)))
        active = [g01, g11, g21]
        queue = [g31]
        while active:
            for g in list(active):
                if not drive(g):
                    active.remove(g)
                    if queue:
                        active.append(queue.pop(0))

    nc.finalize()
    return nc


_NC = None


def _get_nc():
    global _NC
    if _NC is None:
        _NC = build_nc()
    return _NC


def _host_tables():
    import ml_dtypes
    bf16 = ml_dtypes.bfloat16
    inv_freq = 1.0 / (10000.0 ** (np.arange(0, Dh, 2, dtype=np.float32) / Dh))  # [32]
    t = np.arange(T, dtype=np.float32)
    freqs = t[:, None] * inv_freq[None, :]                  # [T, 32]
    emb = np.concatenate([freqs, freqs], axis=-1)           # [T, 64]
    cos = np.cos(emb).T.astype(np.float32)                  # [64, T]
    sin = np.sin(emb).T.astype(np.float32)                  # [64, T]
    sin_signed = sin.copy()
    sin_signed[0:32, :] *= -1.0                             # rotate_half sign fold
    cosT = np.concatenate([cos, cos], axis=0).astype(bf16)  # [128, T] two head-halves
    sinT = np.ascontiguousarray(np.concatenate([sin_signed, sin_signed], axis=0))
    maskT = np.triu(np.ones((128, 128), np.float32)).astype(bf16)  # keep where k <= q
    identT = np.eye(128, dtype=np.float32).astype(bf16)
    sigma = np.empty(64, np.int64)
    sigma[0:32] = 2 * np.arange(32) + 1
    sigma[32:64] = 2 * np.arange(32)
    R = np.zeros((128, 128), np.float32)
    for hh in range(2):
        for d in range(64):
            R[64 * hh + d, 64 * hh + sigma[d]] = 1.0
    rotT = np.ascontiguousarray(R.T).astype(bf16)
    return cosT, sinT, maskT, identT, rotT


def kernel(x, w_qkv, w_out):
    import ml_dtypes
    bf16 = ml_dtypes.bfloat16
    x = np.asarray(x, dtype=np.float32)
    w_qkv = np.asarray(w_qkv, dtype=np.float32)
    w_out = np.asarray(w_out, dtype=np.float32)
    nc = _get_nc()
    cosT, sinT, maskT, identT, rotT = _host_tables()

    in_maps = []
    for core in range(N_CORES):
        b = core // 4
        g = core % 4
        heads = [4 * g + l for l in range(HL)]
        qcols = [w_qkv[:, 64 * h:64 * (h + 1)] for h in heads]
        kcols = [w_qkv[:, C + 64 * h:C + 64 * (h + 1)] for h in heads]
        vcols = [w_qkv[:, 2 * C + 64 * h:2 * C + 64 * (h + 1)] for h in heads]
        # m-tiles: Q01 | K01 | Q23 | K23
        wqkv_loc = np.concatenate(
            [qcols[0], qcols[1], kcols[0], kcols[1], qcols[2], qcols[3], kcols[2], kcols[3]]
            + vcols, axis=1).astype(bf16)                    # [C, 768]
        wo_loc = np.concatenate([w_out[64 * h:64 * (h + 1), :] for h in heads],
                                axis=0).astype(bf16)
        in_maps.append({
            "xt": np.ascontiguousarray(x[b].T).astype(bf16),  # [C, T]
            "wqkv": wqkv_loc,
            "wo": wo_loc,
            "cosT": cosT, "sinT": sinT, "maskT": maskT,
            "identT": identT, "rotT": rotT,
        })

    res = run_bass_kernel_spmd(nc, in_maps, core_ids=list(range(N_CORES)))
    out_arr = np.zeros((B, T, C), np.float32)
    for core in range(N_CORES):
        out_arr[core // 4] += res.results[core]["out"]
    return out_arr
